# revision 9
# baseline (speedup 1.0000x reference)
"""Trainium2 Bass kernel for the MPNN discriminator (NNConv+GRU x6, Set2Set, MLP).

Self-contained: takes FULL inputs, shards across 8 NeuronCores internally,
returns the FULL [512, 2] output.

Strategy (8 cores, SPMD single program, per-core data):
- Graphs split 64-per-core; node ranges follow graph boundaries (node2graph is
  sorted). Edges assigned to the core owning dst, sorted by dst, tiled into
  128-edge tiles with no dst spanning two tiles (host pads with dummy edges
  whose src points at an always-zero h row).
- Edge MLP runs once on device; per-edge weight matrices w_e (en_b2 baked in)
  are materialized to DRAM as bf16 in [e, o*64+i] layout.
- Per layer: indirect-gather h[src] (bf16) -> DVE broadcast-multiply against
  streamed w rows -> grouped reduce over i -> per-edge messages; a host-built
  selection matmul (inv_cnt folded) sums duplicate-dst rows; rows are
  indirect-scattered to a local m table; dense 128-node windows then run
  relu+mask + GRU with PE matmuls in transposed layout; AllGather shares h.
- Set2Set runs fully local (graph-aligned shard) in transposed layout with an
  additive -1e30 mask for the segment softmax; classifier emits [2, 64] per
  core, host concatenates.
"""
import sys
sys.path.insert(0, "/opt/trn_rl_repo")
import numpy as np
import ml_dtypes

N, E, B = 25600, 51200, 512
D, NA, NB, EH = 64, 40, 10, 128
L, ITERS, OUT = 6, 6, 2
NCORES = 8
GPB = B // NCORES
P = 128
DD = D * D
WCH = 512            # psum free-dim chunk
BF = ml_dtypes.bfloat16

_CACHE = {}


# ---------------------------------------------------------------- host plan --
def _plan(src, dst, node2graph):
    nsplit = np.searchsorted(node2graph, np.arange(NCORES + 1) * GPB).astype(np.int64)
    NL = nsplit[1:] - nsplit[:-1]
    NLP = int(np.ceil(NL.max() / P) * P)
    W = NLP // P

    cnt = np.maximum(np.bincount(dst, minlength=N).astype(np.float32), 1.0)
    inv_cnt = (1.0 / cnt).astype(np.float32)
    owner = np.searchsorted(nsplit, dst, side="right") - 1

    per_core = []
    for c in range(NCORES):
        sel = np.where(owner == c)[0]
        order = np.argsort(dst[sel], kind="stable")
        eids = sel[order]
        dl = dst[eids] - nsplit[c]
        tiles, cur = [], []
        i, n = 0, len(eids)
        while i < n:
            j = i
            while j < n and dl[j] == dl[i]:
                j += 1
            if len(cur) + (j - i) > P:
                cur.extend([-1] * (P - len(cur)))
                tiles.append(cur); cur = []
            cur.extend(range(i, j))
            i = j
        if cur:
            cur.extend([-1] * (P - len(cur)))
            tiles.append(cur)
        per_core.append((eids, dl, tiles))

    T_e = max(len(t) for _, _, t in per_core)
    T_e = int(np.ceil(T_e / 4) * 4)          # ET multiple of 512 for chunking
    ET = T_e * P

    cores = []
    for c in range(NCORES):
        eids, dl, tiles = per_core[c]
        while len(tiles) < T_e:
            tiles.append([-1] * P)
        pos = np.array(tiles, dtype=np.int64).reshape(-1)
        valid = pos >= 0
        posc = np.clip(pos, 0, None)
        e_glob = np.where(valid, eids[posc], 0)
        src_idx = np.where(valid, src[e_glob], N).astype(np.int32)
        dst_loc = np.where(valid, dl[posc], 0)
        slot = np.arange(ET) % P
        scat = np.where(valid, dst_loc, NLP + slot).astype(np.int32)
        selm = np.zeros((ET, P), np.float32)
        dmat = scat.reshape(T_e, P)
        vmat = valid.reshape(T_e, P)
        for t in range(T_e):
            eq = dmat[t][:, None] == dmat[t][None, :]
            gd = np.where(vmat[t], dmat[t] + nsplit[c], 0)
            ic = np.where(vmat[t], inv_cnt[gd], 0.0)
            selm[t * P:(t + 1) * P] = eq * ic[None, :]
        xe_order = np.where(valid, e_glob, 0).astype(np.int64)

        gstart = (np.searchsorted(node2graph, np.arange(GPB) + c * GPB) - nsplit[c])
        gend = (np.searchsorted(node2graph, np.arange(GPB) + c * GPB, side="right")
                - nsplit[c])
        amask = np.full((GPB, NLP), -1e30, np.float32)
        for g in range(GPB):
            amask[g, gstart[g]:gend[g]] = 0.0
        lo, hi = nsplit[c], nsplit[c + 1]
        deg = np.bincount(dst[(dst >= lo) & (dst < hi)] - lo, minlength=NLP)
        vmask = (deg[:NLP] > 0).astype(np.float32)
        cores.append(dict(src_idx=src_idx, scat=scat, selm=selm, xe_order=xe_order,
                          amask=amask, vmask=vmask))
    return dict(nsplit=nsplit, NL=NL, NLP=NLP, W=W, T_e=T_e, ET=ET, cores=cores)


# ----------------------------------------------------- walrus wait splitter --
def _split_multi_waits(nc, mybir, bass_rust, max_waits=1):
    for fn in nc.m.functions:
        for bb in fn.blocks:
            insts = bb.instructions
            i = 0
            while i < len(insts):
                ins = insts[i]
                si = ins.sync_info
                if si is not None and si.on_wait and len(si.on_wait) > max_waits:
                    waits = list(si.on_wait)
                    extra, keep = waits[:-max_waits], waits[-max_waits:]
                    si.on_wait = keep
                    for j, w in enumerate(extra):
                        nop = mybir.InstNoOp(name=f"{ins.name}-wsplit{j}")
                        nop.engine = ins.engine
                        nop.sync_info = bass_rust.SyncInfo(on_wait=[w], on_update=[])
                        insts.insert(i, nop)
                        nc.register_instruction(nop, overwrite=True)
                        i += 1
                i += 1


# ----------------------------------------------------------- device program --
def _build(plan_dims):
    import os
    _NL_ = int(os.environ.get("K_LAYERS", "6"))
    _H0_ = os.environ.get("K_H0", "1") == "1"
    _BW_ = os.environ.get("K_BUILD", "1") == "1"
    _S2S_ = os.environ.get("K_S2S", "1") == "1"
    _MSG_ = os.environ.get("K_MSG", "1") == "1"
    _GRU_ = os.environ.get("K_GRU", "1") == "1"
    import bass_rust
    from concourse import bass, mybir
    import concourse.tile as tile

    NLP, W, T_e, ET = (plan_dims["NLP"], plan_dims["W"], plan_dims["T_e"],
                       plan_dims["ET"])
    nsplit = [int(v) for v in plan_dims["nsplit"]]
    NLs = [int(v) for v in plan_dims["NL"]]
    f32, bf16, i32 = mybir.dt.float32, mybir.dt.bfloat16, mybir.dt.int32
    AF = mybir.ActivationFunctionType
    OP = mybir.AluOpType
    AX = mybir.AxisListType
    NCH = [WCH] * (NLP // WCH) + ([NLP % WCH] if NLP % WCH else [])
    ECH = [WCH] * (ET // WCH)                      # ET is a multiple of 512

    nc = bass.Bass(num_swdge_queues=4)

    def din(name, shape, dt=bf16):
        return nc.declare_dram_parameter(name, list(shape), dt, isOutput=False)

    xnT = din("xnT", [NA, N])
    xnl = din("xnl", [NA, NLP])
    xeT = din("xeT", [NB, ET])
    selm = din("selm", [ET, P], f32)
    srcb = din("srcb", [P, T_e], i32)
    scatb = din("scatb", [P, T_e], i32)
    amask = din("amask", [GPB, NLP], f32)
    vmaskb = din("vmaskb", [P, W], f32)
    wemb = din("wemb", [NA, D])
    bnode_rep = din("bnode_rep", [P, D], f32)
    bnode_col = din("bnode_col", [D, 1], f32)
    we_l = din("we_l", [NB, EH])
    be_col = din("be_col", [EH, 1], f32)
    w1_l = din("w1_l", [EH, EH])
    b1_col = din("b1_col", [EH, 1], f32)
    t2p = din("t2p", [EH, DD])
    b2row = din("b2row", [1, DD])
    ones_row = din("ones_row", [1, P])
    wiT = din("wiT", [D, 3 * D])
    whT = din("whT", [D, 3 * D])
    gbias = din("gbias", [D, 4], f32)
    wq0 = din("wq0", [D, 4 * D]); wr0 = din("wr0", [D, 4 * D])
    wh0 = din("wh0", [D, 4 * D]); wi1 = din("wi1", [D, 4 * D])
    wh1 = din("wh1", [D, 4 * D])
    lb0 = din("lb0", [D, 4], f32)
    lb1 = din("lb1", [D, 4], f32)
    c1q = din("c1q", [D, D]); c1r = din("c1r", [D, D])
    c1b = din("c1b", [D, 1], f32)
    c2t = din("c2t", [D, OUT]); c2b = din("c2b", [OUT, 1], f32)
    idf = din("idf", [P, P], f32)
    idb = din("idb", [P, P])
    zf = din("zf", [P, D], f32)
    zb = din("zb", [P, D])
    zmt = din("zmt", [NLP + P, D], f32)
    y = nc.declare_dram_parameter("y", [OUT, GPB], f32, isOutput=True)

    with tile.TileContext(nc) as tc:
        with tc.tile_pool(name="dram", bufs=1, space="DRAM") as dpool, \
             tc.tile_pool(name="const", bufs=1) as cp, \
             tc.tile_pool(name="state", bufs=1) as stp, \
             tc.tile_pool(name="sb", bufs=3) as sb, \
             tc.tile_pool(name="wstream", bufs=3) as wsp, \
             tc.tile_pool(name="prodp", bufs=3) as prp, \
             tc.tile_pool(name="ps", bufs=2, space="PSUM") as ps, \
             tc.tile_pool(name="psg", bufs=4, space="PSUM") as psg:

            h_tab = dpool.tile([N + P, D], bf16)
            w_tab = dpool.tile([ET, DD], bf16)
            m_tab = dpool.tile([NLP + P, D], f32)
            hloc = dpool.tile([NLP, D], bf16)
            hgat = dpool.tile([NCORES * NLP, D], bf16)

            ident = cp.tile([P, P], f32)
            nc.sync.dma_start(out=ident[:], in_=idf[:])
            identb = cp.tile([P, P], bf16)
            nc.sync.dma_start(out=identb[:], in_=idb[:])

            def ld(dram, shape, dt):
                nm = f"c_{dram.name}"
                t = cp.tile(list(shape), dt, name=nm, tag=nm)
                nc.sync.dma_start(out=t[:], in_=dram[:])
                return t

            wemb_s = ld(wemb, [NA, D], bf16)
            bnr_s = ld(bnode_rep, [P, D], f32)
            bnc_s = ld(bnode_col, [D, 1], f32)
            we_s = ld(we_l, [NB, EH], bf16)
            bec_s = ld(be_col, [EH, 1], f32)
            w1_s = ld(w1_l, [EH, EH], bf16)
            b1c_s = ld(b1_col, [EH, 1], f32)
            t2p_s = ld(t2p, [EH, DD], bf16)
            b2r_s = ld(b2row, [1, DD], bf16)
            ones_s = ld(ones_row, [1, P], bf16)
            wiT_s = ld(wiT, [D, 3 * D], bf16)
            whT_s = ld(whT, [D, 3 * D], bf16)
            gb_s = ld(gbias, [D, 4], f32)
            srcb_s = ld(srcb, [P, T_e], i32)
            scatb_s = ld(scatb, [P, T_e], i32)
            vm_s = ld(vmaskb, [P, W], f32)
            am_s = ld(amask, [GPB, NLP], f32)
            wq0_s = ld(wq0, [D, 4 * D], bf16); wr0_s = ld(wr0, [D, 4 * D], bf16)
            wh0_s = ld(wh0, [D, 4 * D], bf16); wi1_s = ld(wi1, [D, 4 * D], bf16)
            wh1_s = ld(wh1, [D, 4 * D], bf16)
            lb0_s = ld(lb0, [D, 4], f32)
            lb1_s = ld(lb1, [D, 4], f32)
            c1q_s = ld(c1q, [D, D], bf16); c1r_s = ld(c1r, [D, D], bf16)
            c1b_s = ld(c1b, [D, 1], f32)
            c2t_s = ld(c2t, [D, OUT], bf16)
            c2b_s = ld(c2b, [OUT, 1], f32)

            hgT = stp.tile([D, NLP], f32)
            hgTb = stp.tile([D, NLP], bf16)
            h_rm = stp.tile([P, W * D], bf16)

            # ---- zero m table + h pad rows (once) ----
            nc.sync.dma_start(out=m_tab[:], in_=zmt[:])
            nc.sync.dma_start(out=h_tab[N:N + P, :], in_=zb[:])

            # ---- h0 row-major (full) -> h_tab ----
            for t in range(N // P if _H0_ else 0):
                xt = sb.tile([NA, P], bf16, tag="xnt")
                nc.sync.dma_start(out=xt[:], in_=xnT[:, t * P:(t + 1) * P])
                h0ps = psg.tile([P, D], f32, tag="psB")
                nc.tensor.matmul(out=h0ps[:], lhsT=xt[:], rhs=wemb_s[:],
                                 start=True, stop=True)
                h0sb = sb.tile([P, D], bf16, tag="h0sb")
                nc.vector.tensor_tensor(out=h0sb[:], in0=h0ps[:], in1=bnr_s[:],
                                        op=OP.add)
                nc.sync.dma_start(out=h_tab[t * P:(t + 1) * P, :], in_=h0sb[:])

            # ---- h0T local -> hgT / hgTb ----
            off = 0
            for ch in NCH:
                xl = sb.tile([NA, WCH], bf16, tag="xnl")
                nc.sync.dma_start(out=xl[:, :ch], in_=xnl[:, off:off + ch])
                hps = ps.tile([D, WCH], f32, tag="psA")
                nc.tensor.matmul(out=hps[:, :ch], lhsT=wemb_s[:],
                                 rhs=xl[:, :ch], start=True, stop=True)
                nc.scalar.activation(out=hgT[:, off:off + ch], in_=hps[:, :ch],
                                     func=AF.Identity, bias=bnc_s[:, :1], scale=1.0)
                nc.vector.tensor_copy(out=hgTb[:, off:off + ch],
                                      in_=hgT[:, off:off + ch])
                off += ch

            # ---- edge MLP -> w_tab (one-time) ----
            for kc in range(len(ECH) if _BW_ else 0):
                xe_sb = sb.tile([NB, WCH], bf16, tag="xe")
                nc.sync.dma_start(out=xe_sb[:], in_=xeT[:, kc * WCH:(kc + 1) * WCH])
                he_ps = ps.tile([EH, WCH], f32, tag="psA")
                nc.tensor.matmul(out=he_ps[:], lhsT=we_s[:], rhs=xe_sb[:],
                                 start=True, stop=True)
                he_sb = sb.tile([EH, WCH], bf16, tag="hesb")
                nc.scalar.activation(out=he_sb[:], in_=he_ps[:], func=AF.Identity,
                                     bias=bec_s[:, :1], scale=1.0)
                u_ps = ps.tile([EH, WCH], f32, tag="psA")
                nc.tensor.matmul(out=u_ps[:], lhsT=w1_s[:], rhs=he_sb[:],
                                 start=True, stop=True)
                u_sb = sb.tile([EH, WCH], bf16, tag="usb")
                nc.scalar.activation(out=u_sb[:], in_=u_ps[:], func=AF.Relu,
                                     bias=b1c_s[:, :1], scale=1.0)
                for tt in range(WCH // P):
                    et = kc * (WCH // P) + tt
                    w_sb = wsp.tile([P, DD], bf16, tag="w")
                    for nb in range(DD // WCH):
                        wps = ps.tile([P, WCH], f32, tag="psA")
                        nc.tensor.matmul(out=wps[:],
                                         lhsT=u_sb[:, tt * P:(tt + 1) * P],
                                         rhs=t2p_s[:, nb * WCH:(nb + 1) * WCH],
                                         start=True, stop=False)
                        nc.tensor.matmul(out=wps[:], lhsT=ones_s[:, :P],
                                         rhs=b2r_s[:, nb * WCH:(nb + 1) * WCH],
                                         start=False, stop=True)
                        if nb % 2 == 0:
                            nc.scalar.copy(out=w_sb[:, nb * WCH:(nb + 1) * WCH],
                                           in_=wps[:])
                        else:
                            nc.vector.tensor_copy(
                                out=w_sb[:, nb * WCH:(nb + 1) * WCH], in_=wps[:])
                    nc.sync.dma_start(out=w_tab[et * P:(et + 1) * P, :], in_=w_sb[:])

            # ================= 6 MPNN layers =================
            for layer in range(min(L, _NL_)):
                for t in range(T_e if _MSG_ else 0):
                    hs = sb.tile([P, D], bf16, tag="hsrc")
                    nc.gpsimd.indirect_dma_start(
                        out=hs[:], out_offset=None, in_=h_tab[:],
                        in_offset=bass.IndirectOffsetOnAxis(
                            ap=srcb_s[:, t:t + 1], axis=0))
                    wt = wsp.tile([P, DD], bf16, tag="w")
                    nc.sync.dma_start(out=wt[:], in_=w_tab[t * P:(t + 1) * P, :])
                    prod = prp.tile([P, DD], bf16, tag="prod")
                    nc.vector.tensor_tensor(
                        out=prod[:].rearrange("p (o i) -> p o i", o=D),
                        in0=wt[:].rearrange("p (o i) -> p o i", o=D),
                        in1=hs[:].unsqueeze(1).broadcast_to([P, D, D]),
                        op=OP.mult)
                    m_e = sb.tile([P, D], f32, tag="me")
                    nc.vector.tensor_reduce(
                        out=m_e[:], in_=prod[:].rearrange("p (o i) -> p o i", o=D),
                        axis=AX.X, op=OP.add)
                    selt = sb.tile([P, P], f32, tag="sel")
                    nc.sync.dma_start(out=selt[:], in_=selm[t * P:(t + 1) * P, :])
                    rows_ps = psg.tile([P, D], f32, tag="psB")
                    nc.tensor.matmul(out=rows_ps[:], lhsT=selt[:], rhs=m_e[:],
                                     start=True, stop=True)
                    rows = sb.tile([P, D], f32, tag="rows")
                    nc.scalar.copy(out=rows[:], in_=rows_ps[:])
                    nc.gpsimd.indirect_dma_start(
                        out=m_tab[:], out_offset=bass.IndirectOffsetOnAxis(
                            ap=scatb_s[:, t:t + 1], axis=0),
                        in_=rows[:], in_offset=None)

                for w in range(W if _GRU_ else 0):
                    mw = sb.tile([P, D], f32, tag="mw")
                    nc.sync.dma_start(out=mw[:], in_=m_tab[w * P:(w + 1) * P, :])
                    mwm = sb.tile([P, D], f32, tag="mwm")
                    nc.scalar.activation(out=mwm[:], in_=mw[:], func=AF.Copy,
                                         scale=vm_s[:, w:w + 1])
                    mt_ps = psg.tile([D, P], f32, tag="psB")
                    nc.tensor.transpose(out=mt_ps[:], in_=mwm[:], identity=ident[:])
                    mtr = sb.tile([D, P], bf16, tag="mtr")
                    nc.scalar.activation(out=mtr[:], in_=mt_ps[:], func=AF.Relu)

                    hgb_w = hgTb[:, w * P:(w + 1) * P]
                    ps_r = psg.tile([D, P], f32, tag="psB")
                    nc.tensor.matmul(out=ps_r[:], lhsT=wiT_s[:, 0:D], rhs=mtr[:],
                                     start=True, stop=False)
                    nc.tensor.matmul(out=ps_r[:], lhsT=whT_s[:, 0:D], rhs=hgb_w,
                                     start=False, stop=True)
                    r_t = sb.tile([D, P], f32, tag="r_t")
                    nc.scalar.activation(out=r_t[:], in_=ps_r[:], func=AF.Sigmoid,
                                         bias=gb_s[:, 0:1], scale=1.0)
                    ps_z = psg.tile([D, P], f32, tag="psB")
                    nc.tensor.matmul(out=ps_z[:], lhsT=wiT_s[:, D:2 * D], rhs=mtr[:],
                                     start=True, stop=False)
                    nc.tensor.matmul(out=ps_z[:], lhsT=whT_s[:, D:2 * D], rhs=hgb_w,
                                     start=False, stop=True)
                    z_t = sb.tile([D, P], f32, tag="z_t")
                    nc.scalar.activation(out=z_t[:], in_=ps_z[:], func=AF.Sigmoid,
                                         bias=gb_s[:, 1:2], scale=1.0)
                    ps_xn = psg.tile([D, P], f32, tag="psB")
                    nc.tensor.matmul(out=ps_xn[:], lhsT=wiT_s[:, 2 * D:3 * D],
                                     rhs=mtr[:], start=True, stop=True)
                    gxn = sb.tile([D, P], f32, tag="gxn")
                    nc.scalar.activation(out=gxn[:], in_=ps_xn[:], func=AF.Identity,
                                         bias=gb_s[:, 2:3], scale=1.0)
                    ps_hn = psg.tile([D, P], f32, tag="psB")
                    nc.tensor.matmul(out=ps_hn[:], lhsT=whT_s[:, 2 * D:3 * D],
                                     rhs=hgb_w, start=True, stop=True)
                    ghn = sb.tile([D, P], f32, tag="ghn")
                    nc.scalar.activation(out=ghn[:], in_=ps_hn[:], func=AF.Identity,
                                         bias=gb_s[:, 3:4], scale=1.0)
                    t1 = sb.tile([D, P], f32, tag="t1")
                    nc.vector.tensor_tensor(out=t1[:], in0=r_t[:], in1=ghn[:],
                                            op=OP.mult)
                    t2 = sb.tile([D, P], f32, tag="t2")
                    nc.vector.tensor_tensor(out=t2[:], in0=t1[:], in1=gxn[:],
                                            op=OP.add)
                    n_t = sb.tile([D, P], f32, tag="n_t")
                    nc.scalar.activation(out=n_t[:], in_=t2[:], func=AF.Tanh)
                    hg_w = hgT[:, w * P:(w + 1) * P]
                    d_t = sb.tile([D, P], f32, tag="d_t")
                    nc.vector.tensor_tensor(out=d_t[:], in0=hg_w, in1=n_t[:],
                                            op=OP.subtract)
                    e_t = sb.tile([D, P], f32, tag="e_t")
                    nc.vector.tensor_tensor(out=e_t[:], in0=z_t[:], in1=d_t[:],
                                            op=OP.mult)
                    nc.vector.tensor_tensor(out=hg_w, in0=e_t[:], in1=n_t[:],
                                            op=OP.add)
                    nc.vector.tensor_copy(out=hgTb[:, w * P:(w + 1) * P], in_=hg_w)
                    hr_ps = psg.tile([P, D], bf16, tag="psB")
                    nc.tensor.transpose(out=hr_ps[:], in_=hgb_w,
                                        identity=identb[:D, :D])
                    if layer < L - 1:
                        hr_sb = sb.tile([P, D], bf16, tag="hr_sb")
                        nc.scalar.copy(out=hr_sb[:], in_=hr_ps[:])
                        nc.sync.dma_start(out=hloc[w * P:(w + 1) * P, :],
                                          in_=hr_sb[:])
                    else:
                        nc.scalar.copy(out=h_rm[:, w * D:(w + 1) * D], in_=hr_ps[:])

                if layer < L - 1:
                    nc.gpsimd.collective_compute(
                        "AllGather", OP.bypass,
                        replica_groups=[list(range(NCORES))],
                        ins=[hloc[:].opt()], outs=[hgat[:].opt()])
                    for c in range(NCORES):
                        nc.sync.dma_start(
                            out=h_tab[nsplit[c]:nsplit[c] + NLs[c], :],
                            in_=hgat[c * NLP:c * NLP + NLs[c], :])

            # ================= Set2Set =================
            qTb = stp.tile([D, GPB], bf16)
            rTb = stp.tile([D, GPB], bf16)
            hT0 = stp.tile([D, GPB], f32)
            cT0 = stp.tile([D, GPB], f32)
            hT1 = stp.tile([D, GPB], f32)
            cT1 = stp.tile([D, GPB], f32)
            h0b = stp.tile([D, GPB], bf16)
            h1b = stp.tile([D, GPB], bf16)
            e_sb = stp.tile([GPB, NLP], f32)
            ee_sb = stp.tile([GPB, NLP], f32)
            al_b = stp.tile([GPB, NLP], bf16)
            for tl in (qTb, rTb, h0b, h1b):
                nc.sync.dma_start(out=tl[:], in_=zb[:D, :GPB])
            for tl in (hT0, cT0, hT1, cT1):
                nc.sync.dma_start(out=tl[:], in_=zf[:D, :GPB])

            def lstm_layer(wx_parts, wh_s, h_b, hT, cT, lb_s, out_b):
                gates = []
                for g in range(4):
                    pst = psg.tile([D, GPB], f32, tag="psB")
                    first = True
                    for (wt_s, rhs_t) in wx_parts:
                        nc.tensor.matmul(out=pst[:],
                                         lhsT=wt_s[:, g * D:(g + 1) * D],
                                         rhs=rhs_t[:], start=first, stop=False)
                        first = False
                    nc.tensor.matmul(out=pst[:], lhsT=wh_s[:, g * D:(g + 1) * D],
                                     rhs=h_b[:], start=False, stop=True)
                    fn = AF.Tanh if g == 2 else AF.Sigmoid
                    gt = sb.tile([D, GPB], f32, tag=f"lstm_g{g}")
                    nc.scalar.activation(out=gt[:], in_=pst[:], func=fn,
                                         bias=lb_s[:, g:g + 1], scale=1.0)
                    gates.append(gt)
                ig, fg, gg, og = gates
                fc = sb.tile([D, GPB], f32, tag="fc")
                nc.vector.tensor_tensor(out=fc[:], in0=fg[:], in1=cT[:], op=OP.mult)
                igg = sb.tile([D, GPB], f32, tag="igg")
                nc.vector.tensor_tensor(out=igg[:], in0=ig[:], in1=gg[:], op=OP.mult)
                nc.vector.tensor_tensor(out=cT[:], in0=fc[:], in1=igg[:], op=OP.add)
                tc_ = sb.tile([D, GPB], f32, tag="tc_")
                nc.scalar.activation(out=tc_[:], in_=cT[:], func=AF.Tanh)
                nc.vector.tensor_tensor(out=hT[:], in0=og[:], in1=tc_[:], op=OP.mult)
                nc.vector.tensor_copy(out=out_b[:], in_=hT[:])

            for it in range(ITERS if _S2S_ else 0):
                lstm_layer([(wq0_s, qTb), (wr0_s, rTb)], wh0_s, h0b, hT0, cT0,
                           lb0_s, h0b)
                lstm_layer([(wi1_s, h0b)], wh1_s, h1b, hT1, cT1, lb1_s, h1b)
                nc.vector.tensor_copy(out=qTb[:], in_=hT1[:])

                off = 0
                for ch in NCH:
                    eps = ps.tile([GPB, WCH], f32, tag="psA")
                    nc.tensor.matmul(out=eps[:, :ch], lhsT=qTb[:],
                                     rhs=hgTb[:, off:off + ch], start=True,
                                     stop=True)
                    nc.vector.tensor_tensor(out=e_sb[:, off:off + ch],
                                            in0=eps[:, :ch],
                                            in1=am_s[:, off:off + ch], op=OP.add)
                    off += ch
                nmax = sb.tile([GPB, 1], f32, tag="nmax")
                nc.vector.tensor_reduce(out=nmax[:], in_=e_sb[:], axis=AX.X,
                                        op=OP.max, negate=True)
                nc.scalar.activation(out=ee_sb[:], in_=e_sb[:], func=AF.Exp,
                                     bias=nmax[:, :1], scale=1.0)
                ssum = sb.tile([GPB, 1], f32, tag="ssum")
                nc.vector.tensor_reduce(out=ssum[:], in_=ee_sb[:], axis=AX.X,
                                        op=OP.add)
                rsum = sb.tile([GPB, 1], f32, tag="rsum")
                nc.vector.reciprocal(out=rsum[:], in_=ssum[:])
                nc.vector.tensor_scalar_mul(al_b[:], ee_sb[:], rsum[:, :1])

                ro_ps = psg.tile([D, GPB], f32, tag="psB")
                for w in range(W):
                    at_ps = psg.tile([P, GPB], bf16, tag="psB")
                    nc.tensor.transpose(out=at_ps[:],
                                        in_=al_b[:, w * P:(w + 1) * P],
                                        identity=identb[:GPB, :GPB])
                    at_b = sb.tile([P, GPB], bf16, tag="at_b")
                    nc.scalar.copy(out=at_b[:], in_=at_ps[:])
                    nc.tensor.matmul(
                        out=ro_ps[:], lhsT=h_rm[:, w * D:(w + 1) * D],
                        rhs=at_b[:], start=(w == 0), stop=(w == W - 1))
                nc.vector.tensor_copy(out=rTb[:], in_=ro_ps[:])

            # ================= classifier =================
            ps1 = psg.tile([D, GPB], f32, tag="psB")
            nc.tensor.matmul(out=ps1[:], lhsT=c1q_s[:], rhs=qTb[:],
                             start=True, stop=False)
            nc.tensor.matmul(out=ps1[:], lhsT=c1r_s[:], rhs=rTb[:],
                             start=False, stop=True)
            z1b = sb.tile([D, GPB], bf16, tag="z1b")
            nc.scalar.activation(out=z1b[:], in_=ps1[:], func=AF.Relu,
                                 bias=c1b_s[:, :1], scale=1.0)
            ps2 = psg.tile([OUT, GPB], f32, tag="psB")
            nc.tensor.matmul(out=ps2[:], lhsT=c2t_s[:], rhs=z1b[:],
                             start=True, stop=True)
            yout = sb.tile([OUT, GPB], f32, tag="yout")
            nc.scalar.activation(out=yout[:], in_=ps2[:], func=AF.Identity,
                                 bias=c2b_s[:, :1], scale=1.0)
            nc.sync.dma_start(out=y[:], in_=yout[:])

    _split_multi_waits(nc, mybir, bass_rust)
    return nc


# ------------------------------------------------------------------- driver --
def kernel(x_node, x_edge, params, src, dst, node2graph):
    from concourse.bass_utils import run_bass_kernel_spmd

    x_node = np.asarray(x_node, np.float32)
    x_edge = np.asarray(x_edge, np.float32)
    src = np.asarray(src, np.int32)
    dst = np.asarray(dst, np.int32)
    node2graph = np.asarray(node2graph, np.int32)
    p = {k: np.asarray(v, np.float32) for k, v in params.items()}

    plan = _plan(src, dst, node2graph)
    NLP, W, T_e, ET = plan["NLP"], plan["W"], plan["T_e"], plan["ET"]
    nsplit = plan["nsplit"]

    key = (NLP, T_e, tuple(int(v) for v in nsplit))
    if key not in _CACHE:
        _CACHE[key] = _build(plan)
    nc = _CACHE[key]

    T2p = p["en_w2"].reshape(D, D, EH).transpose(2, 1, 0).reshape(EH, DD)
    b2p = p["en_b2"].reshape(D, D).T.reshape(1, DD)
    gb = np.stack([
        p["gru_bi"][:D] + p["gru_bh"][:D],
        p["gru_bi"][D:2 * D] + p["gru_bh"][D:2 * D],
        p["gru_bi"][2 * D:],
        p["gru_bh"][2 * D:],
    ], axis=1).astype(np.float32)
    lb0 = (p["lstm_bih0"] + p["lstm_bhh0"]).reshape(4, D).T.copy().astype(np.float32)
    lb1 = (p["lstm_bih1"] + p["lstm_bhh1"]).reshape(4, D).T.copy().astype(np.float32)

    rep = {
        "xnT": np.ascontiguousarray(x_node.T).astype(BF),
        "wemb": np.ascontiguousarray(p["node_emb_w"].T).astype(BF),
        "bnode_rep": np.broadcast_to(p["node_emb_b"], (P, D)).copy().astype(np.float32),
        "bnode_col": p["node_emb_b"].reshape(D, 1).astype(np.float32),
        "we_l": np.ascontiguousarray(p["edge_emb_w"].T).astype(BF),
        "be_col": p["edge_emb_b"].reshape(EH, 1).astype(np.float32),
        "w1_l": np.ascontiguousarray(p["en_w1"].T).astype(BF),
        "b1_col": p["en_b1"].reshape(EH, 1).astype(np.float32),
        "t2p": np.ascontiguousarray(T2p).astype(BF),
        "b2row": np.ascontiguousarray(b2p).astype(BF),
        "ones_row": np.ones((1, P), BF),
        "wiT": np.ascontiguousarray(p["gru_wi"].T).astype(BF),
        "whT": np.ascontiguousarray(p["gru_wh"].T).astype(BF),
        "gbias": gb,
        "wq0": np.ascontiguousarray(p["lstm_wih0"][:, :D].T).astype(BF),
        "wr0": np.ascontiguousarray(p["lstm_wih0"][:, D:].T).astype(BF),
        "wh0": np.ascontiguousarray(p["lstm_whh0"].T).astype(BF),
        "wi1": np.ascontiguousarray(p["lstm_wih1"].T).astype(BF),
        "wh1": np.ascontiguousarray(p["lstm_whh1"].T).astype(BF),
        "lb0": lb0, "lb1": lb1,
        "c1q": np.ascontiguousarray(p["c1_w"][:, :D].T).astype(BF),
        "c1r": np.ascontiguousarray(p["c1_w"][:, D:].T).astype(BF),
        "c1b": p["c1_b"].reshape(D, 1).astype(np.float32),
        "c2t": np.ascontiguousarray(p["c2_w"].T).astype(BF),
        "c2b": p["c2_b"].reshape(OUT, 1).astype(np.float32),
        "idf": np.eye(P, dtype=np.float32),
        "idb": np.eye(P, dtype=np.float32).astype(BF),
        "zf": np.zeros((P, D), np.float32),
        "zb": np.zeros((P, D), BF),
    }

    in_maps = []
    for c in range(NCORES):
        pc = plan["cores"][c]
        lo, hi = int(nsplit[c]), int(nsplit[c + 1])
        xnl = np.zeros((NA, NLP), np.float32)
        xnl[:, :hi - lo] = x_node[lo:hi].T
        m = dict(rep)
        m["zmt"] = np.zeros((NLP + P, D), np.float32)
        m["xnl"] = xnl.astype(BF)
        m["xeT"] = np.ascontiguousarray(x_edge[pc["xe_order"]].T).astype(BF)
        m["selm"] = pc["selm"]
        m["srcb"] = np.ascontiguousarray(pc["src_idx"].reshape(T_e, P).T)
        m["scatb"] = np.ascontiguousarray(pc["scat"].reshape(T_e, P).T)
        m["amask"] = pc["amask"]
        m["vmaskb"] = np.ascontiguousarray(
            pc["vmask"].reshape(W, P).T).astype(np.float32)
        in_maps.append(m)

    res = run_bass_kernel_spmd(nc, in_maps, list(range(NCORES)))
    out = np.concatenate([res.results[c]["y"].T for c in range(NCORES)], axis=0)
    return out.astype(np.float32)


# revision 12
# speedup vs baseline: 1.1099x; 1.1099x over previous
"""Trainium2 Bass kernel for the MPNN discriminator (NNConv+GRU x6, Set2Set, MLP).

Self-contained: takes FULL inputs, shards across 8 NeuronCores internally,
returns the FULL [512, 2] output.

Strategy (8 cores, SPMD single program, per-core data):
- Graphs split 64-per-core; node ranges follow graph boundaries (node2graph is
  sorted). Edges assigned to the core owning dst, sorted by dst, tiled into
  128-edge tiles with no dst spanning two tiles (host pads with dummy edges
  whose src points at an always-zero h row).
- Edge MLP runs once on device; per-edge weight matrices w_e (en_b2 baked in)
  are materialized to DRAM as bf16 in [e, o*64+i] layout.
- Per layer: indirect-gather h[src] (bf16) -> DVE broadcast-multiply against
  streamed w rows -> grouped reduce over i -> per-edge messages; a host-built
  selection matmul (inv_cnt folded) sums duplicate-dst rows; rows are
  indirect-scattered to a local m table; dense 128-node windows then run
  relu+mask + GRU with PE matmuls in transposed layout; AllGather shares h.
- Set2Set runs fully local (graph-aligned shard) in transposed layout with an
  additive -1e30 mask for the segment softmax; classifier emits [2, 64] per
  core, host concatenates.
"""
import sys
sys.path.insert(0, "/opt/trn_rl_repo")
import numpy as np
import ml_dtypes

N, E, B = 25600, 51200, 512
D, NA, NB, EH = 64, 40, 10, 128
L, ITERS, OUT = 6, 6, 2
NCORES = 8
GPB = B // NCORES
P = 128
DD = D * D
WCH = 512            # psum free-dim chunk
BF = ml_dtypes.bfloat16

_CACHE = {}


# ---------------------------------------------------------------- host plan --
def _plan(src, dst, node2graph):
    nsplit = np.searchsorted(node2graph, np.arange(NCORES + 1) * GPB).astype(np.int64)
    NL = nsplit[1:] - nsplit[:-1]
    NLP = int(np.ceil(NL.max() / P) * P)
    W = NLP // P

    cnt = np.maximum(np.bincount(dst, minlength=N).astype(np.float32), 1.0)
    inv_cnt = (1.0 / cnt).astype(np.float32)
    owner = np.searchsorted(nsplit, dst, side="right") - 1

    per_core = []
    for c in range(NCORES):
        sel = np.where(owner == c)[0]
        order = np.argsort(dst[sel], kind="stable")
        eids = sel[order]
        dl = dst[eids] - nsplit[c]
        tiles, cur = [], []
        i, n = 0, len(eids)
        while i < n:
            j = i
            while j < n and dl[j] == dl[i]:
                j += 1
            if len(cur) + (j - i) > P:
                cur.extend([-1] * (P - len(cur)))
                tiles.append(cur); cur = []
            cur.extend(range(i, j))
            i = j
        if cur:
            cur.extend([-1] * (P - len(cur)))
            tiles.append(cur)
        per_core.append((eids, dl, tiles))

    T_e = max(len(t) for _, _, t in per_core)
    T_e = int(np.ceil(T_e / 4) * 4)          # ET multiple of 512 for chunking
    ET = T_e * P

    cores = []
    for c in range(NCORES):
        eids, dl, tiles = per_core[c]
        while len(tiles) < T_e:
            tiles.append([-1] * P)
        pos = np.array(tiles, dtype=np.int64).reshape(-1)
        valid = pos >= 0
        posc = np.clip(pos, 0, None)
        e_glob = np.where(valid, eids[posc], 0)
        src_idx = np.where(valid, src[e_glob], N).astype(np.int32)
        dst_loc = np.where(valid, dl[posc], 0)
        slot = np.arange(ET) % P
        scat = np.where(valid, dst_loc, NLP + slot).astype(np.int32)
        selm = np.zeros((ET, P), np.float32)
        dmat = scat.reshape(T_e, P)
        vmat = valid.reshape(T_e, P)
        for t in range(T_e):
            eq = dmat[t][:, None] == dmat[t][None, :]
            gd = np.where(vmat[t], dmat[t] + nsplit[c], 0)
            ic = np.where(vmat[t], inv_cnt[gd], 0.0)
            selm[t * P:(t + 1) * P] = eq * ic[None, :]
        xe_order = np.where(valid, e_glob, 0).astype(np.int64)

        gstart = (np.searchsorted(node2graph, np.arange(GPB) + c * GPB) - nsplit[c])
        gend = (np.searchsorted(node2graph, np.arange(GPB) + c * GPB, side="right")
                - nsplit[c])
        amask = np.full((GPB, NLP), -1e30, np.float32)
        for g in range(GPB):
            amask[g, gstart[g]:gend[g]] = 0.0
        lo, hi = nsplit[c], nsplit[c + 1]
        deg = np.bincount(dst[(dst >= lo) & (dst < hi)] - lo, minlength=NLP)
        vmask = (deg[:NLP] > 0).astype(np.float32)
        cores.append(dict(src_idx=src_idx, scat=scat, selm=selm, xe_order=xe_order,
                          amask=amask, vmask=vmask))
    return dict(nsplit=nsplit, NL=NL, NLP=NLP, W=W, T_e=T_e, ET=ET, cores=cores)


# ----------------------------------------------------- walrus wait splitter --
def _split_multi_waits(nc, mybir, bass_rust, max_waits=1):
    for fn in nc.m.functions:
        for bb in fn.blocks:
            insts = bb.instructions
            i = 0
            while i < len(insts):
                ins = insts[i]
                si = ins.sync_info
                if si is not None and si.on_wait and len(si.on_wait) > max_waits:
                    waits = list(si.on_wait)
                    extra, keep = waits[:-max_waits], waits[-max_waits:]
                    si.on_wait = keep
                    for j, w in enumerate(extra):
                        nop = mybir.InstNoOp(name=f"{ins.name}-wsplit{j}")
                        nop.engine = ins.engine
                        nop.sync_info = bass_rust.SyncInfo(on_wait=[w], on_update=[])
                        insts.insert(i, nop)
                        nc.register_instruction(nop, overwrite=True)
                        i += 1
                i += 1


# ----------------------------------------------------------- device program --
def _build(plan_dims):
    import os
    _NL_ = int(os.environ.get("K_LAYERS", "6"))
    _H0_ = os.environ.get("K_H0", "1") == "1"
    _BW_ = os.environ.get("K_BUILD", "1") == "1"
    _S2S_ = os.environ.get("K_S2S", "1") == "1"
    _MSG_ = os.environ.get("K_MSG", "1") == "1"
    _GRU_ = os.environ.get("K_GRU", "1") == "1"
    import bass_rust
    from concourse import bass, mybir
    import concourse.tile as tile

    NLP, W, T_e, ET = (plan_dims["NLP"], plan_dims["W"], plan_dims["T_e"],
                       plan_dims["ET"])
    nsplit = [int(v) for v in plan_dims["nsplit"]]
    NLs = [int(v) for v in plan_dims["NL"]]
    f32, bf16, i32 = mybir.dt.float32, mybir.dt.bfloat16, mybir.dt.int32
    AF = mybir.ActivationFunctionType
    OP = mybir.AluOpType
    AX = mybir.AxisListType
    NCH = [WCH] * (NLP // WCH) + ([NLP % WCH] if NLP % WCH else [])
    ECH = [WCH] * (ET // WCH)                      # ET is a multiple of 512

    nc = bass.Bass(num_swdge_queues=4)

    def din(name, shape, dt=bf16):
        return nc.declare_dram_parameter(name, list(shape), dt, isOutput=False)

    xnT = din("xnT", [NA, N])
    xnl = din("xnl", [NA, NLP])
    xeT = din("xeT", [NB, ET])
    selm = din("selm", [ET, P], bf16)
    srcb = din("srcb", [P, T_e], i32)
    scatb = din("scatb", [P, T_e], i32)
    amask = din("amask", [GPB, NLP], bf16)
    vmaskb = din("vmaskb", [P, W], f32)
    wemb = din("wemb", [NA, D])
    bnode_rep = din("bnode_rep", [P, D], f32)
    bnode_col = din("bnode_col", [D, 1], f32)
    we_l = din("we_l", [NB, EH])
    be_col = din("be_col", [EH, 1], f32)
    w1_l = din("w1_l", [EH, EH])
    b1_col = din("b1_col", [EH, 1], f32)
    t2p = din("t2p", [EH, DD])
    b2row = din("b2row", [1, DD])
    ones_row = din("ones_row", [1, P])
    wiT = din("wiT", [D, 3 * D])
    whT = din("whT", [D, 3 * D])
    gbias = din("gbias", [D, 4], f32)
    wq0 = din("wq0", [D, 4 * D]); wr0 = din("wr0", [D, 4 * D])
    wh0 = din("wh0", [D, 4 * D]); wi1 = din("wi1", [D, 4 * D])
    wh1 = din("wh1", [D, 4 * D])
    lb0 = din("lb0", [D, 4], f32)
    lb1 = din("lb1", [D, 4], f32)
    c1q = din("c1q", [D, D]); c1r = din("c1r", [D, D])
    c1b = din("c1b", [D, 1], f32)
    c2t = din("c2t", [D, OUT]); c2b = din("c2b", [OUT, 1], f32)
    idf = din("idf", [P, P], f32)
    idb = din("idb", [P, P])
    zf = din("zf", [P, D], f32)
    zb = din("zb", [P, D])
    zmt = din("zmt", [NLP + P, D], f32)
    y = nc.declare_dram_parameter("y", [OUT, GPB], f32, isOutput=True)

    with tile.TileContext(nc) as tc:
        nc.__enter_lp = nc.allow_low_precision("bf16 message path")
        nc.__enter_lp.__enter__()
        with tc.tile_pool(name="dram", bufs=1, space="DRAM") as dpool, \
             tc.tile_pool(name="const", bufs=1) as cp, \
             tc.tile_pool(name="state", bufs=1) as stp, \
             tc.tile_pool(name="sb", bufs=3) as sb, \
             tc.tile_pool(name="wstream", bufs=3) as wsp, \
             tc.tile_pool(name="prodp", bufs=2) as prp, \
             tc.tile_pool(name="ps", bufs=4, space="PSUM") as ps, \
             tc.tile_pool(name="psg", bufs=4, space="PSUM") as psg:

            h_tab = dpool.tile([N + P, D], bf16)
            w_tab = dpool.tile([ET, DD], bf16)
            m_tab = dpool.tile([NLP + P, D], f32)
            hloc = dpool.tile([NLP, D], bf16)
            hgat = dpool.tile([NCORES * NLP, D], bf16)

            ident = cp.tile([P, P], f32)
            nc.sync.dma_start(out=ident[:], in_=idf[:])
            identb = cp.tile([P, P], bf16)
            nc.sync.dma_start(out=identb[:], in_=idb[:])

            def ld(dram, shape, dt):
                nm = f"c_{dram.name}"
                t = cp.tile(list(shape), dt, name=nm, tag=nm)
                nc.sync.dma_start(out=t[:], in_=dram[:])
                return t

            wemb_s = ld(wemb, [NA, D], bf16)
            bnr_s = ld(bnode_rep, [P, D], f32)
            bnc_s = ld(bnode_col, [D, 1], f32)
            we_s = ld(we_l, [NB, EH], bf16)
            bec_s = ld(be_col, [EH, 1], f32)
            w1_s = ld(w1_l, [EH, EH], bf16)
            b1c_s = ld(b1_col, [EH, 1], f32)
            t2p_s = ld(t2p, [EH, DD], bf16)
            b2r_s = ld(b2row, [1, DD], bf16)
            ones_s = ld(ones_row, [1, P], bf16)
            wiT_s = ld(wiT, [D, 3 * D], bf16)
            whT_s = ld(whT, [D, 3 * D], bf16)
            gb_s = ld(gbias, [D, 4], f32)
            srcb_s = ld(srcb, [P, T_e], i32)
            scatb_s = ld(scatb, [P, T_e], i32)
            vm_s = ld(vmaskb, [P, W], f32)
            am_s = ld(amask, [GPB, NLP], bf16)
            wq0_s = ld(wq0, [D, 4 * D], bf16); wr0_s = ld(wr0, [D, 4 * D], bf16)
            wh0_s = ld(wh0, [D, 4 * D], bf16); wi1_s = ld(wi1, [D, 4 * D], bf16)
            wh1_s = ld(wh1, [D, 4 * D], bf16)
            lb0_s = ld(lb0, [D, 4], f32)
            lb1_s = ld(lb1, [D, 4], f32)
            c1q_s = ld(c1q, [D, D], bf16); c1r_s = ld(c1r, [D, D], bf16)
            c1b_s = ld(c1b, [D, 1], f32)
            c2t_s = ld(c2t, [D, OUT], bf16)
            c2b_s = ld(c2b, [OUT, 1], f32)

            hgT = stp.tile([D, NLP], f32)
            hgTb = stp.tile([D, NLP], bf16)
            h_rm = stp.tile([P, W * D], bf16)
            mtr = stp.tile([D, NLP], bf16)

            # ---- zero m table + h pad rows (once) ----
            nc.sync.dma_start(out=m_tab[:], in_=zmt[:])
            nc.sync.dma_start(out=h_tab[N:N + P, :], in_=zb[:])

            # ---- h0 row-major (full) -> h_tab ----
            for t in range(N // P if _H0_ else 0):
                xt = sb.tile([NA, P], bf16, tag="xnt")
                nc.sync.dma_start(out=xt[:], in_=xnT[:, t * P:(t + 1) * P])
                h0ps = psg.tile([P, D], f32, tag="psB")
                nc.tensor.matmul(out=h0ps[:], lhsT=xt[:], rhs=wemb_s[:],
                                 start=True, stop=True)
                h0sb = sb.tile([P, D], bf16, tag="h0sb")
                nc.vector.tensor_tensor(out=h0sb[:], in0=h0ps[:], in1=bnr_s[:],
                                        op=OP.add)
                nc.sync.dma_start(out=h_tab[t * P:(t + 1) * P, :], in_=h0sb[:])

            # ---- h0T local -> hgT / hgTb ----
            off = 0
            for ch in NCH:
                xl = sb.tile([NA, WCH], bf16, tag="xnl")
                nc.sync.dma_start(out=xl[:, :ch], in_=xnl[:, off:off + ch])
                hps = ps.tile([D, WCH], f32, tag="psA")
                nc.tensor.matmul(out=hps[:, :ch], lhsT=wemb_s[:],
                                 rhs=xl[:, :ch], start=True, stop=True)
                nc.scalar.activation(out=hgT[:, off:off + ch], in_=hps[:, :ch],
                                     func=AF.Identity, bias=bnc_s[:, :1], scale=1.0)
                nc.vector.tensor_copy(out=hgTb[:, off:off + ch],
                                      in_=hgT[:, off:off + ch])
                off += ch

            # ---- edge MLP -> w_tab (one-time) ----
            for kc in range(len(ECH) if _BW_ else 0):
                xe_sb = sb.tile([NB, WCH], bf16, tag="xe")
                nc.sync.dma_start(out=xe_sb[:], in_=xeT[:, kc * WCH:(kc + 1) * WCH])
                he_ps = ps.tile([EH, WCH], f32, tag="psA")
                nc.tensor.matmul(out=he_ps[:], lhsT=we_s[:], rhs=xe_sb[:],
                                 start=True, stop=True)
                he_sb = sb.tile([EH, WCH], bf16, tag="hesb")
                nc.scalar.activation(out=he_sb[:], in_=he_ps[:], func=AF.Identity,
                                     bias=bec_s[:, :1], scale=1.0)
                u_ps = ps.tile([EH, WCH], f32, tag="psA")
                nc.tensor.matmul(out=u_ps[:], lhsT=w1_s[:], rhs=he_sb[:],
                                 start=True, stop=True)
                u_sb = sb.tile([EH, WCH], bf16, tag="usb")
                nc.scalar.activation(out=u_sb[:], in_=u_ps[:], func=AF.Relu,
                                     bias=b1c_s[:, :1], scale=1.0)
                for tt in range(WCH // P):
                    et = kc * (WCH // P) + tt
                    w_sb = wsp.tile([P, DD], bf16, tag="w")
                    for nb in range(DD // WCH):
                        wps = ps.tile([P, WCH], f32, tag="psA")
                        nc.tensor.matmul(out=wps[:],
                                         lhsT=u_sb[:, tt * P:(tt + 1) * P],
                                         rhs=t2p_s[:, nb * WCH:(nb + 1) * WCH],
                                         start=True, stop=False)
                        nc.tensor.matmul(out=wps[:], lhsT=ones_s[:, :P],
                                         rhs=b2r_s[:, nb * WCH:(nb + 1) * WCH],
                                         start=False, stop=True)
                        if nb % 2 == 0:
                            nc.scalar.copy(out=w_sb[:, nb * WCH:(nb + 1) * WCH],
                                           in_=wps[:])
                        else:
                            nc.vector.tensor_copy(
                                out=w_sb[:, nb * WCH:(nb + 1) * WCH], in_=wps[:])
                    nc.sync.dma_start(out=w_tab[et * P:(et + 1) * P, :], in_=w_sb[:])

            # ================= 6 MPNN layers =================
            for layer in range(min(L, _NL_)):
                for t in range(T_e if _MSG_ else 0):
                    hs = sb.tile([P, D], bf16, tag="hsrc")
                    nc.gpsimd.indirect_dma_start(
                        out=hs[:], out_offset=None, in_=h_tab[:],
                        in_offset=bass.IndirectOffsetOnAxis(
                            ap=srcb_s[:, t:t + 1], axis=0))
                    wt = wsp.tile([P, DD], bf16, tag="w")
                    nc.sync.dma_start(out=wt[:], in_=w_tab[t * P:(t + 1) * P, :])
                    prod = prp.tile([P, DD], bf16, tag="prod")
                    nc.vector.tensor_tensor(
                        out=prod[:].rearrange("p (o i) -> p o i", o=D),
                        in0=wt[:].rearrange("p (o i) -> p o i", o=D),
                        in1=hs[:].unsqueeze(1).broadcast_to([P, D, D]),
                        op=OP.mult)
                    pv = prod[:].rearrange("p (o i) -> p o i", o=D)
                    t32 = prp.tile([P, D * 32], bf16, tag="t32")
                    nc.vector.tensor_tensor(
                        out=t32[:].rearrange("p (o i) -> p o i", o=D),
                        in0=pv[:, :, 0:32], in1=pv[:, :, 32:64], op=OP.add)
                    tv = t32[:].rearrange("p (o i) -> p o i", o=D)
                    t16 = prp.tile([P, D * 16], bf16, tag="t16")
                    nc.vector.tensor_tensor(
                        out=t16[:].rearrange("p (o i) -> p o i", o=D),
                        in0=tv[:, :, 0:16], in1=tv[:, :, 16:32], op=OP.add)
                    tv = t16[:].rearrange("p (o i) -> p o i", o=D)
                    t8 = prp.tile([P, D * 8], bf16, tag="t8")
                    nc.vector.tensor_tensor(
                        out=t8[:].rearrange("p (o i) -> p o i", o=D),
                        in0=tv[:, :, 0:8], in1=tv[:, :, 8:16], op=OP.add)
                    m_e = sb.tile([P, D], bf16, tag="me")
                    nc.vector.tensor_reduce(
                        out=m_e[:], in_=t8[:].rearrange("p (o i) -> p o i", o=D),
                        axis=AX.X, op=OP.add)
                    selt = sb.tile([P, P], bf16, tag="sel")
                    nc.sync.dma_start(out=selt[:], in_=selm[t * P:(t + 1) * P, :])
                    rows_ps = psg.tile([P, D], f32, tag="psB")
                    nc.tensor.matmul(out=rows_ps[:], lhsT=selt[:], rhs=m_e[:],
                                     start=True, stop=True)
                    rows = sb.tile([P, D], f32, tag="rows")
                    nc.scalar.copy(out=rows[:], in_=rows_ps[:])
                    nc.gpsimd.indirect_dma_start(
                        out=m_tab[:], out_offset=bass.IndirectOffsetOnAxis(
                            ap=scatb_s[:, t:t + 1], axis=0),
                        in_=rows[:], in_offset=None)

                # phase A: m windows -> relu/mask -> transposed mtr (resident)
                for w in range(W if _GRU_ else 0):
                    mw = sb.tile([P, D], f32, tag="mw")
                    nc.sync.dma_start(out=mw[:], in_=m_tab[w * P:(w + 1) * P, :])
                    mwm = sb.tile([P, D], f32, tag="mwm")
                    nc.scalar.activation(out=mwm[:], in_=mw[:], func=AF.Copy,
                                         scale=vm_s[:, w:w + 1])
                    mt_ps = psg.tile([D, P], f32, tag="psB")
                    nc.tensor.transpose(out=mt_ps[:], in_=mwm[:], identity=ident[:])
                    nc.scalar.activation(out=mtr[:, w * P:(w + 1) * P],
                                         in_=mt_ps[:], func=AF.Relu)
                # phase B: batched GRU over 512-wide chunks
                off = 0
                for ch in (NCH if _GRU_ else []):
                    sl = slice(off, off + ch)
                    mch = mtr[:, sl]
                    hch = hgTb[:, sl]
                    ps_r = ps.tile([D, WCH], f32, tag="psA")
                    nc.tensor.matmul(out=ps_r[:, :ch], lhsT=wiT_s[:, 0:D],
                                     rhs=mch, start=True, stop=False)
                    nc.tensor.matmul(out=ps_r[:, :ch], lhsT=whT_s[:, 0:D],
                                     rhs=hch, start=False, stop=True)
                    r_t = sb.tile([D, WCH], f32, tag="r_t", bufs=2)
                    nc.scalar.activation(out=r_t[:, :ch], in_=ps_r[:, :ch],
                                         func=AF.Sigmoid, bias=gb_s[:, 0:1],
                                         scale=1.0)
                    ps_z = ps.tile([D, WCH], f32, tag="psA")
                    nc.tensor.matmul(out=ps_z[:, :ch], lhsT=wiT_s[:, D:2 * D],
                                     rhs=mch, start=True, stop=False)
                    nc.tensor.matmul(out=ps_z[:, :ch], lhsT=whT_s[:, D:2 * D],
                                     rhs=hch, start=False, stop=True)
                    z_t = sb.tile([D, WCH], f32, tag="z_t", bufs=2)
                    nc.scalar.activation(out=z_t[:, :ch], in_=ps_z[:, :ch],
                                         func=AF.Sigmoid, bias=gb_s[:, 1:2],
                                         scale=1.0)
                    ps_xn = ps.tile([D, WCH], f32, tag="psA")
                    nc.tensor.matmul(out=ps_xn[:, :ch], lhsT=wiT_s[:, 2 * D:3 * D],
                                     rhs=mch, start=True, stop=True)
                    gxn = sb.tile([D, WCH], f32, tag="gxn", bufs=2)
                    nc.scalar.activation(out=gxn[:, :ch], in_=ps_xn[:, :ch],
                                         func=AF.Identity, bias=gb_s[:, 2:3],
                                         scale=1.0)
                    ps_hn = ps.tile([D, WCH], f32, tag="psA")
                    nc.tensor.matmul(out=ps_hn[:, :ch], lhsT=whT_s[:, 2 * D:3 * D],
                                     rhs=hch, start=True, stop=True)
                    ghn = sb.tile([D, WCH], f32, tag="ghn", bufs=2)
                    nc.scalar.activation(out=ghn[:, :ch], in_=ps_hn[:, :ch],
                                         func=AF.Identity, bias=gb_s[:, 3:4],
                                         scale=1.0)
                    t1 = sb.tile([D, WCH], f32, tag="t1", bufs=2)
                    nc.vector.tensor_tensor(out=t1[:, :ch], in0=r_t[:, :ch],
                                            in1=ghn[:, :ch], op=OP.mult)
                    nc.vector.tensor_tensor(out=t1[:, :ch], in0=t1[:, :ch],
                                            in1=gxn[:, :ch], op=OP.add)
                    n_t = sb.tile([D, WCH], f32, tag="n_t", bufs=2)
                    nc.scalar.activation(out=n_t[:, :ch], in_=t1[:, :ch],
                                         func=AF.Tanh)
                    hgch = hgT[:, sl]
                    nc.vector.tensor_tensor(out=t1[:, :ch], in0=hgch,
                                            in1=n_t[:, :ch], op=OP.subtract)
                    nc.vector.tensor_tensor(out=t1[:, :ch], in0=z_t[:, :ch],
                                            in1=t1[:, :ch], op=OP.mult)
                    nc.vector.tensor_tensor(out=hgch, in0=t1[:, :ch],
                                            in1=n_t[:, :ch], op=OP.add)
                    nc.vector.tensor_copy(out=hgTb[:, sl], in_=hgch)
                    off += ch
                # phase C: transposed h back to row-major for sharing/readout
                for w in range(W if _GRU_ else 0):
                    hgb_w = hgTb[:, w * P:(w + 1) * P]
                    hr_ps = psg.tile([P, D], bf16, tag="psB")
                    nc.tensor.transpose(out=hr_ps[:], in_=hgb_w,
                                        identity=identb[:D, :D])
                    if layer < L - 1:
                        hr_sb = sb.tile([P, D], bf16, tag="hr_sb")
                        nc.scalar.copy(out=hr_sb[:], in_=hr_ps[:])
                        nc.sync.dma_start(out=hloc[w * P:(w + 1) * P, :],
                                          in_=hr_sb[:])
                    else:
                        nc.scalar.copy(out=h_rm[:, w * D:(w + 1) * D], in_=hr_ps[:])

                if layer < L - 1:
                    nc.gpsimd.collective_compute(
                        "AllGather", OP.bypass,
                        replica_groups=[list(range(NCORES))],
                        ins=[hloc[:].opt()], outs=[hgat[:].opt()])
                    for c in range(NCORES):
                        nc.sync.dma_start(
                            out=h_tab[nsplit[c]:nsplit[c] + NLs[c], :],
                            in_=hgat[c * NLP:c * NLP + NLs[c], :])

            # ================= Set2Set =================
            qTb = stp.tile([D, GPB], bf16)
            rTb = stp.tile([D, GPB], bf16)
            hT0 = stp.tile([D, GPB], f32)
            cT0 = stp.tile([D, GPB], f32)
            hT1 = stp.tile([D, GPB], f32)
            cT1 = stp.tile([D, GPB], f32)
            h0b = stp.tile([D, GPB], bf16)
            h1b = stp.tile([D, GPB], bf16)
            e_sb = stp.tile([GPB, NLP], f32)
            al_b = stp.tile([GPB, NLP], bf16)
            for tl in (qTb, rTb, h0b, h1b):
                nc.sync.dma_start(out=tl[:], in_=zb[:D, :GPB])
            for tl in (hT0, cT0, hT1, cT1):
                nc.sync.dma_start(out=tl[:], in_=zf[:D, :GPB])

            def lstm_layer(wx_parts, wh_s, h_b, hT, cT, lb_s, out_b):
                gates = []
                for g in range(4):
                    pst = psg.tile([D, GPB], f32, tag="psB")
                    first = True
                    for (wt_s, rhs_t) in wx_parts:
                        nc.tensor.matmul(out=pst[:],
                                         lhsT=wt_s[:, g * D:(g + 1) * D],
                                         rhs=rhs_t[:], start=first, stop=False)
                        first = False
                    nc.tensor.matmul(out=pst[:], lhsT=wh_s[:, g * D:(g + 1) * D],
                                     rhs=h_b[:], start=False, stop=True)
                    fn = AF.Tanh if g == 2 else AF.Sigmoid
                    gt = sb.tile([D, GPB], f32, tag=f"lstm_g{g}")
                    nc.scalar.activation(out=gt[:], in_=pst[:], func=fn,
                                         bias=lb_s[:, g:g + 1], scale=1.0)
                    gates.append(gt)
                ig, fg, gg, og = gates
                fc = sb.tile([D, GPB], f32, tag="fc")
                nc.vector.tensor_tensor(out=fc[:], in0=fg[:], in1=cT[:], op=OP.mult)
                igg = sb.tile([D, GPB], f32, tag="igg")
                nc.vector.tensor_tensor(out=igg[:], in0=ig[:], in1=gg[:], op=OP.mult)
                nc.vector.tensor_tensor(out=cT[:], in0=fc[:], in1=igg[:], op=OP.add)
                tc_ = sb.tile([D, GPB], f32, tag="tc_")
                nc.scalar.activation(out=tc_[:], in_=cT[:], func=AF.Tanh)
                nc.vector.tensor_tensor(out=hT[:], in0=og[:], in1=tc_[:], op=OP.mult)
                nc.vector.tensor_copy(out=out_b[:], in_=hT[:])

            for it in range(ITERS if _S2S_ else 0):
                lstm_layer([(wq0_s, qTb), (wr0_s, rTb)], wh0_s, h0b, hT0, cT0,
                           lb0_s, h0b)
                lstm_layer([(wi1_s, h0b)], wh1_s, h1b, hT1, cT1, lb1_s, h1b)
                nc.vector.tensor_copy(out=qTb[:], in_=hT1[:])

                off = 0
                for ch in NCH:
                    eps = ps.tile([GPB, WCH], f32, tag="psA")
                    nc.tensor.matmul(out=eps[:, :ch], lhsT=qTb[:],
                                     rhs=hgTb[:, off:off + ch], start=True,
                                     stop=True)
                    nc.vector.tensor_tensor(out=e_sb[:, off:off + ch],
                                            in0=eps[:, :ch],
                                            in1=am_s[:, off:off + ch], op=OP.add)
                    off += ch
                nmax = sb.tile([GPB, 1], f32, tag="nmax")
                nc.vector.tensor_reduce(out=nmax[:], in_=e_sb[:], axis=AX.X,
                                        op=OP.max, negate=True)
                nc.scalar.activation(out=e_sb[:], in_=e_sb[:], func=AF.Exp,
                                     bias=nmax[:, :1], scale=1.0)
                ssum = sb.tile([GPB, 1], f32, tag="ssum")
                nc.vector.tensor_reduce(out=ssum[:], in_=e_sb[:], axis=AX.X,
                                        op=OP.add)
                rsum = sb.tile([GPB, 1], f32, tag="rsum")
                nc.vector.reciprocal(out=rsum[:], in_=ssum[:])
                nc.vector.tensor_scalar_mul(al_b[:], e_sb[:], rsum[:, :1])

                ro_ps = psg.tile([D, GPB], f32, tag="psB")
                for w in range(W):
                    at_ps = psg.tile([P, GPB], bf16, tag="psB")
                    nc.tensor.transpose(out=at_ps[:],
                                        in_=al_b[:, w * P:(w + 1) * P],
                                        identity=identb[:GPB, :GPB])
                    at_b = sb.tile([P, GPB], bf16, tag="at_b")
                    nc.scalar.copy(out=at_b[:], in_=at_ps[:])
                    nc.tensor.matmul(
                        out=ro_ps[:], lhsT=h_rm[:, w * D:(w + 1) * D],
                        rhs=at_b[:], start=(w == 0), stop=(w == W - 1))
                nc.vector.tensor_copy(out=rTb[:], in_=ro_ps[:])

            # ================= classifier =================
            ps1 = psg.tile([D, GPB], f32, tag="psB")
            nc.tensor.matmul(out=ps1[:], lhsT=c1q_s[:], rhs=qTb[:],
                             start=True, stop=False)
            nc.tensor.matmul(out=ps1[:], lhsT=c1r_s[:], rhs=rTb[:],
                             start=False, stop=True)
            z1b = sb.tile([D, GPB], bf16, tag="z1b")
            nc.scalar.activation(out=z1b[:], in_=ps1[:], func=AF.Relu,
                                 bias=c1b_s[:, :1], scale=1.0)
            ps2 = psg.tile([OUT, GPB], f32, tag="psB")
            nc.tensor.matmul(out=ps2[:], lhsT=c2t_s[:], rhs=z1b[:],
                             start=True, stop=True)
            yout = sb.tile([OUT, GPB], f32, tag="yout")
            nc.scalar.activation(out=yout[:], in_=ps2[:], func=AF.Identity,
                                 bias=c2b_s[:, :1], scale=1.0)
            nc.sync.dma_start(out=y[:], in_=yout[:])
        nc.__enter_lp.__exit__(None, None, None)

    _split_multi_waits(nc, mybir, bass_rust)
    return nc


# ------------------------------------------------------------------- driver --
def kernel(x_node, x_edge, params, src, dst, node2graph):
    from concourse.bass_utils import run_bass_kernel_spmd

    x_node = np.asarray(x_node, np.float32)
    x_edge = np.asarray(x_edge, np.float32)
    src = np.asarray(src, np.int32)
    dst = np.asarray(dst, np.int32)
    node2graph = np.asarray(node2graph, np.int32)
    p = {k: np.asarray(v, np.float32) for k, v in params.items()}

    plan = _plan(src, dst, node2graph)
    NLP, W, T_e, ET = plan["NLP"], plan["W"], plan["T_e"], plan["ET"]
    nsplit = plan["nsplit"]

    key = (NLP, T_e, tuple(int(v) for v in nsplit))
    if key not in _CACHE:
        _CACHE[key] = _build(plan)
    nc = _CACHE[key]

    T2p = p["en_w2"].reshape(D, D, EH).transpose(2, 1, 0).reshape(EH, DD)
    b2p = p["en_b2"].reshape(D, D).T.reshape(1, DD)
    gb = np.stack([
        p["gru_bi"][:D] + p["gru_bh"][:D],
        p["gru_bi"][D:2 * D] + p["gru_bh"][D:2 * D],
        p["gru_bi"][2 * D:],
        p["gru_bh"][2 * D:],
    ], axis=1).astype(np.float32)
    lb0 = (p["lstm_bih0"] + p["lstm_bhh0"]).reshape(4, D).T.copy().astype(np.float32)
    lb1 = (p["lstm_bih1"] + p["lstm_bhh1"]).reshape(4, D).T.copy().astype(np.float32)

    rep = {
        "xnT": np.ascontiguousarray(x_node.T).astype(BF),
        "wemb": np.ascontiguousarray(p["node_emb_w"].T).astype(BF),
        "bnode_rep": np.broadcast_to(p["node_emb_b"], (P, D)).copy().astype(np.float32),
        "bnode_col": p["node_emb_b"].reshape(D, 1).astype(np.float32),
        "we_l": np.ascontiguousarray(p["edge_emb_w"].T).astype(BF),
        "be_col": p["edge_emb_b"].reshape(EH, 1).astype(np.float32),
        "w1_l": np.ascontiguousarray(p["en_w1"].T).astype(BF),
        "b1_col": p["en_b1"].reshape(EH, 1).astype(np.float32),
        "t2p": np.ascontiguousarray(T2p).astype(BF),
        "b2row": np.ascontiguousarray(b2p).astype(BF),
        "ones_row": np.ones((1, P), BF),
        "wiT": np.ascontiguousarray(p["gru_wi"].T).astype(BF),
        "whT": np.ascontiguousarray(p["gru_wh"].T).astype(BF),
        "gbias": gb,
        "wq0": np.ascontiguousarray(p["lstm_wih0"][:, :D].T).astype(BF),
        "wr0": np.ascontiguousarray(p["lstm_wih0"][:, D:].T).astype(BF),
        "wh0": np.ascontiguousarray(p["lstm_whh0"].T).astype(BF),
        "wi1": np.ascontiguousarray(p["lstm_wih1"].T).astype(BF),
        "wh1": np.ascontiguousarray(p["lstm_whh1"].T).astype(BF),
        "lb0": lb0, "lb1": lb1,
        "c1q": np.ascontiguousarray(p["c1_w"][:, :D].T).astype(BF),
        "c1r": np.ascontiguousarray(p["c1_w"][:, D:].T).astype(BF),
        "c1b": p["c1_b"].reshape(D, 1).astype(np.float32),
        "c2t": np.ascontiguousarray(p["c2_w"].T).astype(BF),
        "c2b": p["c2_b"].reshape(OUT, 1).astype(np.float32),
        "idf": np.eye(P, dtype=np.float32),
        "idb": np.eye(P, dtype=np.float32).astype(BF),
        "zf": np.zeros((P, D), np.float32),
        "zb": np.zeros((P, D), BF),
    }

    in_maps = []
    for c in range(NCORES):
        pc = plan["cores"][c]
        lo, hi = int(nsplit[c]), int(nsplit[c + 1])
        xnl = np.zeros((NA, NLP), np.float32)
        xnl[:, :hi - lo] = x_node[lo:hi].T
        m = dict(rep)
        m["zmt"] = np.zeros((NLP + P, D), np.float32)
        m["xnl"] = xnl.astype(BF)
        m["xeT"] = np.ascontiguousarray(x_edge[pc["xe_order"]].T).astype(BF)
        m["selm"] = pc["selm"].astype(BF)
        m["srcb"] = np.ascontiguousarray(pc["src_idx"].reshape(T_e, P).T)
        m["scatb"] = np.ascontiguousarray(pc["scat"].reshape(T_e, P).T)
        m["amask"] = pc["amask"].astype(BF)
        m["vmaskb"] = np.ascontiguousarray(
            pc["vmask"].reshape(W, P).T).astype(np.float32)
        in_maps.append(m)

    res = run_bass_kernel_spmd(nc, in_maps, list(range(NCORES)))
    out = np.concatenate([res.results[c]["y"].T for c in range(NCORES)], axis=0)
    return out.astype(np.float32)


# revision 15
# speedup vs baseline: 1.2600x; 1.1353x over previous
"""Trainium2 Bass kernel for the MPNN discriminator (NNConv+GRU x6, Set2Set, MLP).

Self-contained: takes FULL inputs, shards across 8 NeuronCores internally,
returns the FULL [512, 2] output.

Strategy (8 cores, SPMD single program, per-core data):
- Graphs split 64-per-core; node ranges follow graph boundaries (node2graph is
  sorted). Edges assigned to the core owning dst, sorted by dst, tiled into
  128-edge tiles with no dst spanning two tiles (host pads with dummy edges
  whose src points at an always-zero h row).
- Edge MLP runs once on device; per-edge weight matrices w_e (en_b2 baked in)
  are materialized to DRAM as bf16 in [e, o*64+i] layout.
- Per layer: indirect-gather h[src] (bf16) -> DVE broadcast-multiply against
  streamed w rows -> grouped reduce over i -> per-edge messages; a host-built
  selection matmul (inv_cnt folded) sums duplicate-dst rows; rows are
  indirect-scattered to a local m table; dense 128-node windows then run
  relu+mask + GRU with PE matmuls in transposed layout; AllGather shares h.
- Set2Set runs fully local (graph-aligned shard) in transposed layout with an
  additive -1e30 mask for the segment softmax; classifier emits [2, 64] per
  core, host concatenates.
"""
import sys
sys.path.insert(0, "/opt/trn_rl_repo")
import numpy as np
import ml_dtypes

N, E, B = 25600, 51200, 512
D, NA, NB, EH = 64, 40, 10, 128
L, ITERS, OUT = 6, 6, 2
NCORES = 8
GPB = B // NCORES
P = 128
DD = D * D
WCH = 512            # psum free-dim chunk
BF = ml_dtypes.bfloat16

_CACHE = {}


# ---------------------------------------------------------------- host plan --
def _plan(src, dst, node2graph):
    nsplit = np.searchsorted(node2graph, np.arange(NCORES + 1) * GPB).astype(np.int64)
    NL = nsplit[1:] - nsplit[:-1]
    NLP = int(np.ceil(NL.max() / P) * P)
    W = NLP // P

    cnt = np.maximum(np.bincount(dst, minlength=N).astype(np.float32), 1.0)
    inv_cnt = (1.0 / cnt).astype(np.float32)
    owner = np.searchsorted(nsplit, dst, side="right") - 1

    per_core = []
    for c in range(NCORES):
        sel = np.where(owner == c)[0]
        order = np.argsort(dst[sel], kind="stable")
        eids = sel[order]
        dl = dst[eids] - nsplit[c]
        tiles, cur = [], []
        i, n = 0, len(eids)
        while i < n:
            j = i
            while j < n and dl[j] == dl[i]:
                j += 1
            if len(cur) + (j - i) > P:
                cur.extend([-1] * (P - len(cur)))
                tiles.append(cur); cur = []
            cur.extend(range(i, j))
            i = j
        if cur:
            cur.extend([-1] * (P - len(cur)))
            tiles.append(cur)
        per_core.append((eids, dl, tiles))

    T_e = max(len(t) for _, _, t in per_core)
    T_e = int(np.ceil(T_e / 4) * 4)          # ET multiple of 512 for chunking
    ET = T_e * P

    cores = []
    for c in range(NCORES):
        eids, dl, tiles = per_core[c]
        while len(tiles) < T_e:
            tiles.append([-1] * P)
        pos = np.array(tiles, dtype=np.int64).reshape(-1)
        valid = pos >= 0
        posc = np.clip(pos, 0, None)
        e_glob = np.where(valid, eids[posc], 0)
        # padded h-table coords: row = owner*NLP + (src - nsplit[owner]);
        # dummy rows contribute zero via the selection matrix, so any
        # finite row works -- use 0.
        sown = np.searchsorted(nsplit, src[e_glob], side="right") - 1
        spad = sown * NLP + (src[e_glob] - nsplit[sown])
        src_idx = np.where(valid, spad, 0).astype(np.int32)
        dst_loc = np.where(valid, dl[posc], 0)
        slot = np.arange(ET) % P
        scat = np.where(valid, dst_loc, NLP + slot).astype(np.int32)
        selm = np.zeros((ET, P), np.float32)
        dmat = scat.reshape(T_e, P)
        vmat = valid.reshape(T_e, P)
        for t in range(T_e):
            eq = dmat[t][:, None] == dmat[t][None, :]
            gd = np.where(vmat[t], dmat[t] + nsplit[c], 0)
            ic = np.where(vmat[t], inv_cnt[gd], 0.0)
            selm[t * P:(t + 1) * P] = eq * ic[None, :]
        xe_order = np.where(valid, e_glob, 0).astype(np.int64)

        gstart = (np.searchsorted(node2graph, np.arange(GPB) + c * GPB) - nsplit[c])
        gend = (np.searchsorted(node2graph, np.arange(GPB) + c * GPB, side="right")
                - nsplit[c])
        amask = np.full((GPB, NLP), -1e30, np.float32)
        for g in range(GPB):
            amask[g, gstart[g]:gend[g]] = 0.0
        lo, hi = nsplit[c], nsplit[c + 1]
        deg = np.bincount(dst[(dst >= lo) & (dst < hi)] - lo, minlength=NLP)
        vmask = (deg[:NLP] > 0).astype(np.float32)
        cores.append(dict(src_idx=src_idx, scat=scat, selm=selm, xe_order=xe_order,
                          amask=amask, vmask=vmask))
    return dict(nsplit=nsplit, NL=NL, NLP=NLP, W=W, T_e=T_e, ET=ET, cores=cores)


# ----------------------------------------------------- walrus wait splitter --
def _split_multi_waits(nc, mybir, bass_rust, max_waits=1):
    for fn in nc.m.functions:
        for bb in fn.blocks:
            insts = bb.instructions
            i = 0
            while i < len(insts):
                ins = insts[i]
                si = ins.sync_info
                if si is not None and si.on_wait and len(si.on_wait) > max_waits:
                    waits = list(si.on_wait)
                    extra, keep = waits[:-max_waits], waits[-max_waits:]
                    si.on_wait = keep
                    for j, w in enumerate(extra):
                        nop = mybir.InstNoOp(name=f"{ins.name}-wsplit{j}")
                        nop.engine = ins.engine
                        nop.sync_info = bass_rust.SyncInfo(on_wait=[w], on_update=[])
                        insts.insert(i, nop)
                        nc.register_instruction(nop, overwrite=True)
                        i += 1
                i += 1


# ----------------------------------------------------------- device program --
def _build(plan_dims):
    import os
    _NL_ = int(os.environ.get("K_LAYERS", "6"))
    _H0_ = os.environ.get("K_H0", "1") == "1"
    _BW_ = os.environ.get("K_BUILD", "1") == "1"
    _S2S_ = os.environ.get("K_S2S", "1") == "1"
    _MSG_ = os.environ.get("K_MSG", "1") == "1"
    _GRU_ = os.environ.get("K_GRU", "1") == "1"
    import bass_rust
    from concourse import bass, mybir
    import concourse.tile as tile

    NLP, W, T_e, ET = (plan_dims["NLP"], plan_dims["W"], plan_dims["T_e"],
                       plan_dims["ET"])
    nsplit = [int(v) for v in plan_dims["nsplit"]]
    NLs = [int(v) for v in plan_dims["NL"]]
    f32, bf16, i32 = mybir.dt.float32, mybir.dt.bfloat16, mybir.dt.int32
    AF = mybir.ActivationFunctionType
    OP = mybir.AluOpType
    AX = mybir.AxisListType
    NCH = [WCH] * (NLP // WCH) + ([NLP % WCH] if NLP % WCH else [])
    ECH = [WCH] * (ET // WCH)                      # ET is a multiple of 512

    nc = bass.Bass(num_swdge_queues=4)

    def din(name, shape, dt=bf16):
        return nc.declare_dram_parameter(name, list(shape), dt, isOutput=False)

    xnp = din("xnp", [NA, NCORES * NLP])
    xnl = din("xnl", [NA, NLP])
    xeT = din("xeT", [NB, ET])
    selm = din("selm", [ET, P], bf16)
    srcb = din("srcb", [P, T_e], i32)
    scatb = din("scatb", [P, T_e], i32)
    amask = din("amask", [GPB, NLP], bf16)
    vmaskb = din("vmaskb", [P, W], f32)
    wemb = din("wemb", [NA, D])
    bnode_rep = din("bnode_rep", [P, D], f32)
    bnode_col = din("bnode_col", [D, 1], f32)
    we_l = din("we_l", [NB, EH])
    be_col = din("be_col", [EH, 1], f32)
    w1_l = din("w1_l", [EH, EH])
    b1_col = din("b1_col", [EH, 1], f32)
    t2p = din("t2p", [EH, DD])
    b2rep = din("b2rep", [P, DD])
    wiT = din("wiT", [D, 3 * D])
    whT = din("whT", [D, 3 * D])
    gbias = din("gbias", [D, 4], f32)
    wq0 = din("wq0", [D, 4 * D]); wr0 = din("wr0", [D, 4 * D])
    wh0 = din("wh0", [D, 4 * D]); wi1 = din("wi1", [D, 4 * D])
    wh1 = din("wh1", [D, 4 * D])
    lb0 = din("lb0", [D, 4], f32)
    lb1 = din("lb1", [D, 4], f32)
    c1q = din("c1q", [D, D]); c1r = din("c1r", [D, D])
    c1b = din("c1b", [D, 1], f32)
    c2t = din("c2t", [D, OUT]); c2b = din("c2b", [OUT, 1], f32)
    idf = din("idf", [P, P], f32)
    idb = din("idb", [P, P])
    zf = din("zf", [P, D], f32)
    zb = din("zb", [P, D])
    zmt = din("zmt", [NLP + P, D], f32)
    y = nc.declare_dram_parameter("y", [OUT, GPB], f32, isOutput=True)

    with tile.TileContext(nc) as tc:
        nc.__enter_lp = nc.allow_low_precision("bf16 message path")
        nc.__enter_lp.__enter__()
        with tc.tile_pool(name="dram", bufs=1, space="DRAM") as dpool, \
             tc.tile_pool(name="const", bufs=1) as cp, \
             tc.tile_pool(name="state", bufs=1) as stp, \
             tc.tile_pool(name="sb", bufs=3) as sb, \
             tc.tile_pool(name="wstream", bufs=3) as wsp, \
             tc.tile_pool(name="prodp", bufs=2) as prp, \
             tc.tile_pool(name="ps", bufs=4, space="PSUM") as ps, \
             tc.tile_pool(name="psg", bufs=4, space="PSUM") as psg:

            w_tab = dpool.tile([ET, DD], bf16)
            m_tab = dpool.tile([NLP + P, D], f32)
            hloc = dpool.tile([NLP, D], bf16)
            hgat = dpool.tile([NCORES * NLP, D], bf16)

            ident = cp.tile([P, P], f32)
            nc.sync.dma_start(out=ident[:], in_=idf[:])
            identb = cp.tile([P, P], bf16)
            nc.sync.dma_start(out=identb[:], in_=idb[:])

            def ld(dram, shape, dt):
                nm = f"c_{dram.name}"
                t = cp.tile(list(shape), dt, name=nm, tag=nm)
                nc.sync.dma_start(out=t[:], in_=dram[:])
                return t

            wemb_s = ld(wemb, [NA, D], bf16)
            bnr_s = ld(bnode_rep, [P, D], f32)
            bnc_s = ld(bnode_col, [D, 1], f32)
            we_s = ld(we_l, [NB, EH], bf16)
            bec_s = ld(be_col, [EH, 1], f32)
            w1_s = ld(w1_l, [EH, EH], bf16)
            b1c_s = ld(b1_col, [EH, 1], f32)
            t2p_s = ld(t2p, [EH, DD], bf16)
            b2r_s = wsp.tile([P, DD], bf16, name="b2rep_s", tag="w")
            nc.sync.dma_start(out=b2r_s[:], in_=b2rep[:])
            wiT_s = ld(wiT, [D, 3 * D], bf16)
            whT_s = ld(whT, [D, 3 * D], bf16)
            gb_s = ld(gbias, [D, 4], f32)
            srcb_s = ld(srcb, [P, T_e], i32)
            scatb_s = ld(scatb, [P, T_e], i32)
            vm_s = ld(vmaskb, [P, W], f32)
            am_s = ld(amask, [GPB, NLP], bf16)
            wq0_s = ld(wq0, [D, 4 * D], bf16); wr0_s = ld(wr0, [D, 4 * D], bf16)
            wh0_s = ld(wh0, [D, 4 * D], bf16); wi1_s = ld(wi1, [D, 4 * D], bf16)
            wh1_s = ld(wh1, [D, 4 * D], bf16)
            lb0_s = ld(lb0, [D, 4], f32)
            lb1_s = ld(lb1, [D, 4], f32)
            c1q_s = ld(c1q, [D, D], bf16); c1r_s = ld(c1r, [D, D], bf16)
            c1b_s = ld(c1b, [D, 1], f32)
            c2t_s = ld(c2t, [D, OUT], bf16)
            c2b_s = ld(c2b, [OUT, 1], f32)

            hgT = stp.tile([D, NLP], f32)
            hgTb = stp.tile([D, NLP], bf16)
            h_rm = stp.tile([P, W * D], bf16)
            mtr = stp.tile([D, NLP], bf16)

            # ---- zero m table (once) ----
            nc.scalar.dma_start(out=m_tab[:], in_=zmt[:])

            # ---- h0 full (padded coords) -> hgat ----
            NT = NCORES * NLP
            for kc in range(NT // WCH if _H0_ else 0):
                xt = sb.tile([NA, WCH], bf16, tag="xnt")
                nc.scalar.dma_start(out=xt[:], in_=xnp[:, kc * WCH:(kc + 1) * WCH])
                h0ps = ps.tile([D, WCH], f32, tag="psA")
                nc.tensor.matmul(out=h0ps[:], lhsT=wemb_s[:], rhs=xt[:],
                                 start=True, stop=True)
                h0tb = sb.tile([D, WCH], bf16, tag="h0tb")
                nc.scalar.activation(out=h0tb[:], in_=h0ps[:], func=AF.Identity,
                                     bias=bnc_s[:, :1], scale=1.0)
                h0st = sb.tile([P, (WCH // P) * D], bf16, tag="h0st")
                for tt in range(WCH // P):
                    trp = psg.tile([P, D], bf16, tag="psB")
                    nc.tensor.transpose(out=trp[:],
                                        in_=h0tb[:, tt * P:(tt + 1) * P],
                                        identity=identb[:D, :D])
                    nc.scalar.copy(out=h0st[:, tt * D:(tt + 1) * D], in_=trp[:])
                nc.scalar.dma_start(
                    out=hgat[kc * WCH:(kc + 1) * WCH, :].rearrange(
                        "(t p) d -> p t d", p=P),
                    in_=h0st[:].rearrange("p (t d) -> p t d", d=D))

            # ---- h0T local -> hgT / hgTb ----
            off = 0
            for ch in NCH:
                xl = sb.tile([NA, WCH], bf16, tag="xnl")
                nc.sync.dma_start(out=xl[:, :ch], in_=xnl[:, off:off + ch])
                hps = ps.tile([D, WCH], f32, tag="psA")
                nc.tensor.matmul(out=hps[:, :ch], lhsT=wemb_s[:],
                                 rhs=xl[:, :ch], start=True, stop=True)
                nc.scalar.activation(out=hgT[:, off:off + ch], in_=hps[:, :ch],
                                     func=AF.Identity, bias=bnc_s[:, :1], scale=1.0)
                nc.vector.tensor_copy(out=hgTb[:, off:off + ch],
                                      in_=hgT[:, off:off + ch])
                off += ch

            # ---- edge MLP -> w_tab (one-time) ----
            for kc in range(len(ECH) if _BW_ else 0):
                xe_sb = sb.tile([NB, WCH], bf16, tag="xe")
                nc.sync.dma_start(out=xe_sb[:], in_=xeT[:, kc * WCH:(kc + 1) * WCH])
                he_ps = ps.tile([EH, WCH], f32, tag="psA")
                nc.tensor.matmul(out=he_ps[:], lhsT=we_s[:], rhs=xe_sb[:],
                                 start=True, stop=True)
                he_sb = sb.tile([EH, WCH], bf16, tag="hesb")
                nc.scalar.activation(out=he_sb[:], in_=he_ps[:], func=AF.Identity,
                                     bias=bec_s[:, :1], scale=1.0)
                u_ps = ps.tile([EH, WCH], f32, tag="psA")
                nc.tensor.matmul(out=u_ps[:], lhsT=w1_s[:], rhs=he_sb[:],
                                 start=True, stop=True)
                u_sb = sb.tile([EH, WCH], bf16, tag="usb")
                nc.scalar.activation(out=u_sb[:], in_=u_ps[:], func=AF.Relu,
                                     bias=b1c_s[:, :1], scale=1.0)
                for tt in range(WCH // P):
                    et = kc * (WCH // P) + tt
                    w_sb = wsp.tile([P, DD], bf16, tag="w")
                    for nb in range(DD // WCH):
                        wps = ps.tile([P, WCH], f32, tag="psA")
                        nc.tensor.matmul(out=wps[:],
                                         lhsT=u_sb[:, tt * P:(tt + 1) * P],
                                         rhs=t2p_s[:, nb * WCH:(nb + 1) * WCH],
                                         start=True, stop=True)
                        nc.vector.tensor_tensor(
                            out=w_sb[:, nb * WCH:(nb + 1) * WCH], in0=wps[:],
                            in1=b2r_s[:, nb * WCH:(nb + 1) * WCH], op=OP.add)
                    nc.sync.dma_start(out=w_tab[et * P:(et + 1) * P, :], in_=w_sb[:])

            # ================= 6 MPNN layers =================
            for layer in range(min(L, _NL_)):
                for t in range(T_e if _MSG_ else 0):
                    hs = sb.tile([P, D], bf16, tag="hsrc")
                    nc.gpsimd.indirect_dma_start(
                        out=hs[:], out_offset=None, in_=hgat[:],
                        in_offset=bass.IndirectOffsetOnAxis(
                            ap=srcb_s[:, t:t + 1], axis=0))
                    wt = wsp.tile([P, DD], bf16, tag="w")
                    nc.sync.dma_start(out=wt[:], in_=w_tab[t * P:(t + 1) * P, :])
                    prod = prp.tile([P, DD], bf16, tag="prod")
                    nc.vector.tensor_tensor(
                        out=prod[:].rearrange("p (o i) -> p o i", o=D),
                        in0=wt[:].rearrange("p (o i) -> p o i", o=D),
                        in1=hs[:].unsqueeze(1).broadcast_to([P, D, D]),
                        op=OP.mult)
                    pv = prod[:].rearrange("p (o i) -> p o i", o=D)
                    t32 = prp.tile([P, D * 32], bf16, tag="t32")
                    nc.vector.tensor_tensor(
                        out=t32[:].rearrange("p (o i) -> p o i", o=D),
                        in0=pv[:, :, 0:32], in1=pv[:, :, 32:64], op=OP.add)
                    tv = t32[:].rearrange("p (o i) -> p o i", o=D)
                    t16 = prp.tile([P, D * 16], bf16, tag="t16")
                    nc.vector.tensor_tensor(
                        out=t16[:].rearrange("p (o i) -> p o i", o=D),
                        in0=tv[:, :, 0:16], in1=tv[:, :, 16:32], op=OP.add)
                    tv = t16[:].rearrange("p (o i) -> p o i", o=D)
                    t8 = prp.tile([P, D * 8], bf16, tag="t8")
                    nc.vector.tensor_tensor(
                        out=t8[:].rearrange("p (o i) -> p o i", o=D),
                        in0=tv[:, :, 0:8], in1=tv[:, :, 8:16], op=OP.add)
                    m_e = sb.tile([P, D], bf16, tag="me")
                    nc.vector.tensor_reduce(
                        out=m_e[:], in_=t8[:].rearrange("p (o i) -> p o i", o=D),
                        axis=AX.X, op=OP.add)
                    selt = sb.tile([P, P], bf16, tag="sel")
                    nc.sync.dma_start(out=selt[:], in_=selm[t * P:(t + 1) * P, :])
                    rows_ps = psg.tile([P, D], f32, tag="psB")
                    nc.tensor.matmul(out=rows_ps[:], lhsT=selt[:], rhs=m_e[:],
                                     start=True, stop=True)
                    rows = sb.tile([P, D], f32, tag="rows")
                    nc.scalar.copy(out=rows[:], in_=rows_ps[:])
                    nc.gpsimd.indirect_dma_start(
                        out=m_tab[:], out_offset=bass.IndirectOffsetOnAxis(
                            ap=scatb_s[:, t:t + 1], axis=0),
                        in_=rows[:], in_offset=None)

                # phase A: m windows -> relu/mask -> transposed mtr (resident)
                for w in range(W if _GRU_ else 0):
                    mw = sb.tile([P, D], f32, tag="mw")
                    nc.sync.dma_start(out=mw[:], in_=m_tab[w * P:(w + 1) * P, :])
                    mwm = sb.tile([P, D], f32, tag="mwm")
                    nc.scalar.activation(out=mwm[:], in_=mw[:], func=AF.Copy,
                                         scale=vm_s[:, w:w + 1])
                    mt_ps = psg.tile([D, P], f32, tag="psB")
                    nc.tensor.transpose(out=mt_ps[:], in_=mwm[:], identity=ident[:])
                    nc.scalar.activation(out=mtr[:, w * P:(w + 1) * P],
                                         in_=mt_ps[:], func=AF.Relu)
                # phase B: batched GRU over 512-wide chunks
                off = 0
                for ch in (NCH if _GRU_ else []):
                    sl = slice(off, off + ch)
                    mch = mtr[:, sl]
                    hch = hgTb[:, sl]
                    ps_r = ps.tile([D, WCH], f32, tag="psA")
                    nc.tensor.matmul(out=ps_r[:, :ch], lhsT=wiT_s[:, 0:D],
                                     rhs=mch, start=True, stop=False)
                    nc.tensor.matmul(out=ps_r[:, :ch], lhsT=whT_s[:, 0:D],
                                     rhs=hch, start=False, stop=True)
                    r_t = sb.tile([D, WCH], f32, tag="r_t", bufs=2)
                    nc.scalar.activation(out=r_t[:, :ch], in_=ps_r[:, :ch],
                                         func=AF.Sigmoid, bias=gb_s[:, 0:1],
                                         scale=1.0)
                    ps_z = ps.tile([D, WCH], f32, tag="psA")
                    nc.tensor.matmul(out=ps_z[:, :ch], lhsT=wiT_s[:, D:2 * D],
                                     rhs=mch, start=True, stop=False)
                    nc.tensor.matmul(out=ps_z[:, :ch], lhsT=whT_s[:, D:2 * D],
                                     rhs=hch, start=False, stop=True)
                    z_t = sb.tile([D, WCH], f32, tag="z_t", bufs=2)
                    nc.scalar.activation(out=z_t[:, :ch], in_=ps_z[:, :ch],
                                         func=AF.Sigmoid, bias=gb_s[:, 1:2],
                                         scale=1.0)
                    ps_xn = ps.tile([D, WCH], f32, tag="psA")
                    nc.tensor.matmul(out=ps_xn[:, :ch], lhsT=wiT_s[:, 2 * D:3 * D],
                                     rhs=mch, start=True, stop=True)
                    gxn = sb.tile([D, WCH], f32, tag="gxn", bufs=2)
                    nc.scalar.activation(out=gxn[:, :ch], in_=ps_xn[:, :ch],
                                         func=AF.Identity, bias=gb_s[:, 2:3],
                                         scale=1.0)
                    ps_hn = ps.tile([D, WCH], f32, tag="psA")
                    nc.tensor.matmul(out=ps_hn[:, :ch], lhsT=whT_s[:, 2 * D:3 * D],
                                     rhs=hch, start=True, stop=True)
                    ghn = sb.tile([D, WCH], f32, tag="ghn", bufs=2)
                    nc.scalar.activation(out=ghn[:, :ch], in_=ps_hn[:, :ch],
                                         func=AF.Identity, bias=gb_s[:, 3:4],
                                         scale=1.0)
                    t1 = sb.tile([D, WCH], f32, tag="t1", bufs=2)
                    nc.vector.tensor_tensor(out=t1[:, :ch], in0=r_t[:, :ch],
                                            in1=ghn[:, :ch], op=OP.mult)
                    nc.vector.tensor_tensor(out=t1[:, :ch], in0=t1[:, :ch],
                                            in1=gxn[:, :ch], op=OP.add)
                    n_t = sb.tile([D, WCH], f32, tag="n_t", bufs=2)
                    nc.scalar.activation(out=n_t[:, :ch], in_=t1[:, :ch],
                                         func=AF.Tanh)
                    hgch = hgT[:, sl]
                    nc.vector.tensor_tensor(out=t1[:, :ch], in0=hgch,
                                            in1=n_t[:, :ch], op=OP.subtract)
                    nc.vector.tensor_tensor(out=t1[:, :ch], in0=z_t[:, :ch],
                                            in1=t1[:, :ch], op=OP.mult)
                    nc.vector.tensor_tensor(out=hgch, in0=t1[:, :ch],
                                            in1=n_t[:, :ch], op=OP.add)
                    nc.vector.tensor_copy(out=hgTb[:, sl], in_=hgch)
                    off += ch
                # phase C: transposed h back to row-major for sharing/readout
                for w in range(W if _GRU_ else 0):
                    hgb_w = hgTb[:, w * P:(w + 1) * P]
                    hr_ps = psg.tile([P, D], bf16, tag="psB")
                    nc.tensor.transpose(out=hr_ps[:], in_=hgb_w,
                                        identity=identb[:D, :D])
                    if layer < L - 1:
                        hr_sb = sb.tile([P, D], bf16, tag="hr_sb")
                        nc.scalar.copy(out=hr_sb[:], in_=hr_ps[:])
                        nc.sync.dma_start(out=hloc[w * P:(w + 1) * P, :],
                                          in_=hr_sb[:])
                    else:
                        nc.scalar.copy(out=h_rm[:, w * D:(w + 1) * D], in_=hr_ps[:])

                if layer < L - 1:
                    nc.gpsimd.collective_compute(
                        "AllGather", OP.bypass,
                        replica_groups=[list(range(NCORES))],
                        ins=[hloc[:].opt()], outs=[hgat[:].opt()])

            # ================= Set2Set =================
            qTb = stp.tile([D, GPB], bf16)
            rTb = stp.tile([D, GPB], bf16)
            hT0 = stp.tile([D, GPB], f32)
            cT0 = stp.tile([D, GPB], f32)
            hT1 = stp.tile([D, GPB], f32)
            cT1 = stp.tile([D, GPB], f32)
            h0b = stp.tile([D, GPB], bf16)
            h1b = stp.tile([D, GPB], bf16)
            e_sb = stp.tile([GPB, NLP], f32)
            al_b = stp.tile([GPB, NLP], bf16)
            for tl in (qTb, rTb, h0b, h1b):
                nc.scalar.dma_start(out=tl[:], in_=zb[:D, :GPB])
            for tl in (hT0, cT0, hT1, cT1):
                nc.scalar.dma_start(out=tl[:], in_=zf[:D, :GPB])

            def lstm_layer(wx_parts, wh_s, h_b, hT, cT, lb_s, out_b):
                gates = []
                for g in range(4):
                    pst = psg.tile([D, GPB], f32, tag="psB")
                    first = True
                    for (wt_s, rhs_t) in wx_parts:
                        nc.tensor.matmul(out=pst[:],
                                         lhsT=wt_s[:, g * D:(g + 1) * D],
                                         rhs=rhs_t[:], start=first, stop=False)
                        first = False
                    nc.tensor.matmul(out=pst[:], lhsT=wh_s[:, g * D:(g + 1) * D],
                                     rhs=h_b[:], start=False, stop=True)
                    fn = AF.Tanh if g == 2 else AF.Sigmoid
                    gt = sb.tile([D, GPB], f32, tag=f"lstm_g{g}")
                    nc.scalar.activation(out=gt[:], in_=pst[:], func=fn,
                                         bias=lb_s[:, g:g + 1], scale=1.0)
                    gates.append(gt)
                ig, fg, gg, og = gates
                fc = sb.tile([D, GPB], f32, tag="fc")
                nc.vector.tensor_tensor(out=fc[:], in0=fg[:], in1=cT[:], op=OP.mult)
                igg = sb.tile([D, GPB], f32, tag="igg")
                nc.vector.tensor_tensor(out=igg[:], in0=ig[:], in1=gg[:], op=OP.mult)
                nc.vector.tensor_tensor(out=cT[:], in0=fc[:], in1=igg[:], op=OP.add)
                tc_ = sb.tile([D, GPB], f32, tag="tc_")
                nc.scalar.activation(out=tc_[:], in_=cT[:], func=AF.Tanh)
                nc.vector.tensor_tensor(out=hT[:], in0=og[:], in1=tc_[:], op=OP.mult)
                nc.vector.tensor_copy(out=out_b[:], in_=hT[:])

            for it in range(ITERS if _S2S_ else 0):
                lstm_layer([(wq0_s, qTb), (wr0_s, rTb)], wh0_s, h0b, hT0, cT0,
                           lb0_s, h0b)
                lstm_layer([(wi1_s, h0b)], wh1_s, h1b, hT1, cT1, lb1_s, h1b)
                nc.vector.tensor_copy(out=qTb[:], in_=hT1[:])

                off = 0
                for ch in NCH:
                    eps = ps.tile([GPB, WCH], f32, tag="psA")
                    nc.tensor.matmul(out=eps[:, :ch], lhsT=qTb[:],
                                     rhs=hgTb[:, off:off + ch], start=True,
                                     stop=True)
                    nc.vector.tensor_tensor(out=e_sb[:, off:off + ch],
                                            in0=eps[:, :ch],
                                            in1=am_s[:, off:off + ch], op=OP.add)
                    off += ch
                nmax = sb.tile([GPB, 1], f32, tag="nmax")
                nc.vector.tensor_reduce(out=nmax[:], in_=e_sb[:], axis=AX.X,
                                        op=OP.max, negate=True)
                nc.scalar.activation(out=e_sb[:], in_=e_sb[:], func=AF.Exp,
                                     bias=nmax[:, :1], scale=1.0)
                ssum = sb.tile([GPB, 1], f32, tag="ssum")
                nc.vector.tensor_reduce(out=ssum[:], in_=e_sb[:], axis=AX.X,
                                        op=OP.add)
                rsum = sb.tile([GPB, 1], f32, tag="rsum")
                nc.vector.reciprocal(out=rsum[:], in_=ssum[:])
                nc.vector.tensor_scalar_mul(al_b[:], e_sb[:], rsum[:, :1])

                ro_ps = psg.tile([D, GPB], f32, tag="psB")
                for w in range(W):
                    at_ps = psg.tile([P, GPB], bf16, tag="psB")
                    nc.tensor.transpose(out=at_ps[:],
                                        in_=al_b[:, w * P:(w + 1) * P],
                                        identity=identb[:GPB, :GPB])
                    at_b = sb.tile([P, GPB], bf16, tag="at_b")
                    nc.scalar.copy(out=at_b[:], in_=at_ps[:])
                    nc.tensor.matmul(
                        out=ro_ps[:], lhsT=h_rm[:, w * D:(w + 1) * D],
                        rhs=at_b[:], start=(w == 0), stop=(w == W - 1))
                nc.vector.tensor_copy(out=rTb[:], in_=ro_ps[:])

            # ================= classifier =================
            ps1 = psg.tile([D, GPB], f32, tag="psB")
            nc.tensor.matmul(out=ps1[:], lhsT=c1q_s[:], rhs=qTb[:],
                             start=True, stop=False)
            nc.tensor.matmul(out=ps1[:], lhsT=c1r_s[:], rhs=rTb[:],
                             start=False, stop=True)
            z1b = sb.tile([D, GPB], bf16, tag="z1b")
            nc.scalar.activation(out=z1b[:], in_=ps1[:], func=AF.Relu,
                                 bias=c1b_s[:, :1], scale=1.0)
            ps2 = psg.tile([OUT, GPB], f32, tag="psB")
            nc.tensor.matmul(out=ps2[:], lhsT=c2t_s[:], rhs=z1b[:],
                             start=True, stop=True)
            yout = sb.tile([OUT, GPB], f32, tag="yout")
            nc.scalar.activation(out=yout[:], in_=ps2[:], func=AF.Identity,
                                 bias=c2b_s[:, :1], scale=1.0)
            nc.sync.dma_start(out=y[:], in_=yout[:])
        nc.__enter_lp.__exit__(None, None, None)

    _split_multi_waits(nc, mybir, bass_rust)
    return nc


# ------------------------------------------------------------------- driver --
def kernel(x_node, x_edge, params, src, dst, node2graph):
    from concourse.bass_utils import run_bass_kernel_spmd

    x_node = np.asarray(x_node, np.float32)
    x_edge = np.asarray(x_edge, np.float32)
    src = np.asarray(src, np.int32)
    dst = np.asarray(dst, np.int32)
    node2graph = np.asarray(node2graph, np.int32)
    p = {k: np.asarray(v, np.float32) for k, v in params.items()}

    plan = _plan(src, dst, node2graph)
    NLP, W, T_e, ET = plan["NLP"], plan["W"], plan["T_e"], plan["ET"]
    nsplit = plan["nsplit"]

    key = (NLP, T_e, tuple(int(v) for v in nsplit))
    if key not in _CACHE:
        _CACHE[key] = _build(plan)
    nc = _CACHE[key]

    T2p = p["en_w2"].reshape(D, D, EH).transpose(2, 1, 0).reshape(EH, DD)
    b2p = p["en_b2"].reshape(D, D).T.reshape(1, DD)
    gb = np.stack([
        p["gru_bi"][:D] + p["gru_bh"][:D],
        p["gru_bi"][D:2 * D] + p["gru_bh"][D:2 * D],
        p["gru_bi"][2 * D:],
        p["gru_bh"][2 * D:],
    ], axis=1).astype(np.float32)
    lb0 = (p["lstm_bih0"] + p["lstm_bhh0"]).reshape(4, D).T.copy().astype(np.float32)
    lb1 = (p["lstm_bih1"] + p["lstm_bhh1"]).reshape(4, D).T.copy().astype(np.float32)

    NT = NCORES * NLP
    xnp = np.zeros((NA, NT), np.float32)
    for c in range(NCORES):
        lo, hi = int(nsplit[c]), int(nsplit[c + 1])
        xnp[:, c * NLP:c * NLP + hi - lo] = x_node[lo:hi].T
    b2rep_np = np.broadcast_to(b2p, (P, DD)).copy()
    rep = {
        "xnp": xnp.astype(BF),
        "wemb": np.ascontiguousarray(p["node_emb_w"].T).astype(BF),
        "bnode_rep": np.broadcast_to(p["node_emb_b"], (P, D)).copy().astype(np.float32),
        "bnode_col": p["node_emb_b"].reshape(D, 1).astype(np.float32),
        "we_l": np.ascontiguousarray(p["edge_emb_w"].T).astype(BF),
        "be_col": p["edge_emb_b"].reshape(EH, 1).astype(np.float32),
        "w1_l": np.ascontiguousarray(p["en_w1"].T).astype(BF),
        "b1_col": p["en_b1"].reshape(EH, 1).astype(np.float32),
        "t2p": np.ascontiguousarray(T2p).astype(BF),
        "b2rep": b2rep_np.astype(BF),
        "wiT": np.ascontiguousarray(p["gru_wi"].T).astype(BF),
        "whT": np.ascontiguousarray(p["gru_wh"].T).astype(BF),
        "gbias": gb,
        "wq0": np.ascontiguousarray(p["lstm_wih0"][:, :D].T).astype(BF),
        "wr0": np.ascontiguousarray(p["lstm_wih0"][:, D:].T).astype(BF),
        "wh0": np.ascontiguousarray(p["lstm_whh0"].T).astype(BF),
        "wi1": np.ascontiguousarray(p["lstm_wih1"].T).astype(BF),
        "wh1": np.ascontiguousarray(p["lstm_whh1"].T).astype(BF),
        "lb0": lb0, "lb1": lb1,
        "c1q": np.ascontiguousarray(p["c1_w"][:, :D].T).astype(BF),
        "c1r": np.ascontiguousarray(p["c1_w"][:, D:].T).astype(BF),
        "c1b": p["c1_b"].reshape(D, 1).astype(np.float32),
        "c2t": np.ascontiguousarray(p["c2_w"].T).astype(BF),
        "c2b": p["c2_b"].reshape(OUT, 1).astype(np.float32),
        "idf": np.eye(P, dtype=np.float32),
        "idb": np.eye(P, dtype=np.float32).astype(BF),
        "zf": np.zeros((P, D), np.float32),
        "zb": np.zeros((P, D), BF),
    }

    in_maps = []
    for c in range(NCORES):
        pc = plan["cores"][c]
        lo, hi = int(nsplit[c]), int(nsplit[c + 1])
        xnl = np.zeros((NA, NLP), np.float32)
        xnl[:, :hi - lo] = x_node[lo:hi].T
        m = dict(rep)
        m["zmt"] = np.zeros((NLP + P, D), np.float32)
        m["xnl"] = xnl.astype(BF)
        m["xeT"] = np.ascontiguousarray(x_edge[pc["xe_order"]].T).astype(BF)
        m["selm"] = pc["selm"].astype(BF)
        m["srcb"] = np.ascontiguousarray(pc["src_idx"].reshape(T_e, P).T)
        m["scatb"] = np.ascontiguousarray(pc["scat"].reshape(T_e, P).T)
        m["amask"] = pc["amask"].astype(BF)
        m["vmaskb"] = np.ascontiguousarray(
            pc["vmask"].reshape(W, P).T).astype(np.float32)
        in_maps.append(m)

    res = run_bass_kernel_spmd(nc, in_maps, list(range(NCORES)))
    out = np.concatenate([res.results[c]["y"].T for c in range(NCORES)], axis=0)
    return out.astype(np.float32)


# revision 17
# speedup vs baseline: 1.2640x; 1.0032x over previous
"""Trainium2 Bass kernel for the MPNN discriminator (NNConv+GRU x6, Set2Set, MLP).

Self-contained: takes FULL inputs, shards across 8 NeuronCores internally,
returns the FULL [512, 2] output.

Strategy (8 cores, SPMD single program, per-core data):
- Graphs split 64-per-core; node ranges follow graph boundaries (node2graph is
  sorted). Edges assigned to the core owning dst, sorted by dst, tiled into
  128-edge tiles with no dst spanning two tiles (host pads with dummy edges
  whose src points at an always-zero h row).
- Edge MLP runs once on device; per-edge weight matrices w_e (en_b2 baked in)
  are materialized to DRAM as bf16 in [e, o*64+i] layout.
- Per layer: indirect-gather h[src] (bf16) -> DVE broadcast-multiply against
  streamed w rows -> grouped reduce over i -> per-edge messages; a host-built
  selection matmul (inv_cnt folded) sums duplicate-dst rows; rows are
  indirect-scattered to a local m table; dense 128-node windows then run
  relu+mask + GRU with PE matmuls in transposed layout; AllGather shares h.
- Set2Set runs fully local (graph-aligned shard) in transposed layout with an
  additive -1e30 mask for the segment softmax; classifier emits [2, 64] per
  core, host concatenates.
"""
import sys
sys.path.insert(0, "/opt/trn_rl_repo")
import numpy as np
import ml_dtypes

N, E, B = 25600, 51200, 512
D, NA, NB, EH = 64, 40, 10, 128
L, ITERS, OUT = 6, 6, 2
NCORES = 8
GPB = B // NCORES
P = 128
DD = D * D
WCH = 512            # psum free-dim chunk
BF = ml_dtypes.bfloat16

_CACHE = {}


# ---------------------------------------------------------------- host plan --
def _plan(src, dst, node2graph):
    nsplit = np.searchsorted(node2graph, np.arange(NCORES + 1) * GPB).astype(np.int64)
    NL = nsplit[1:] - nsplit[:-1]
    NLP = int(np.ceil(NL.max() / P) * P)
    W = NLP // P

    cnt = np.maximum(np.bincount(dst, minlength=N).astype(np.float32), 1.0)
    inv_cnt = (1.0 / cnt).astype(np.float32)
    owner = np.searchsorted(nsplit, dst, side="right") - 1

    per_core = []
    for c in range(NCORES):
        sel = np.where(owner == c)[0]
        order = np.argsort(dst[sel], kind="stable")
        eids = sel[order]
        dl = dst[eids] - nsplit[c]
        tiles, cur = [], []
        i, n = 0, len(eids)
        while i < n:
            j = i
            while j < n and dl[j] == dl[i]:
                j += 1
            if len(cur) + (j - i) > P:
                cur.extend([-1] * (P - len(cur)))
                tiles.append(cur); cur = []
            cur.extend(range(i, j))
            i = j
        if cur:
            cur.extend([-1] * (P - len(cur)))
            tiles.append(cur)
        per_core.append((eids, dl, tiles))

    T_e = max(len(t) for _, _, t in per_core)
    T_e = int(np.ceil(T_e / 4) * 4)          # ET multiple of 512 for chunking
    ET = T_e * P

    cores = []
    for c in range(NCORES):
        eids, dl, tiles = per_core[c]
        while len(tiles) < T_e:
            tiles.append([-1] * P)
        pos = np.array(tiles, dtype=np.int64).reshape(-1)
        valid = pos >= 0
        posc = np.clip(pos, 0, None)
        e_glob = np.where(valid, eids[posc], 0)
        # padded h-table coords: row = owner*NLP + (src - nsplit[owner]);
        # dummy rows contribute zero via the selection matrix, so any
        # finite row works -- use 0.
        sown = np.searchsorted(nsplit, src[e_glob], side="right") - 1
        spad = sown * NLP + (src[e_glob] - nsplit[sown])
        src_idx = np.where(valid, spad, 0).astype(np.int32)
        dst_loc = np.where(valid, dl[posc], 0)
        slot = np.arange(ET) % P
        scat = np.where(valid, dst_loc, NLP + slot).astype(np.int32)
        selm = np.zeros((ET, P), np.float32)
        dmat = scat.reshape(T_e, P)
        vmat = valid.reshape(T_e, P)
        for t in range(T_e):
            eq = dmat[t][:, None] == dmat[t][None, :]
            gd = np.where(vmat[t], dmat[t] + nsplit[c], 0)
            ic = np.where(vmat[t], inv_cnt[gd], 0.0)
            selm[t * P:(t + 1) * P] = eq * ic[None, :]
        xe_order = np.where(valid, e_glob, 0).astype(np.int64)

        gstart = (np.searchsorted(node2graph, np.arange(GPB) + c * GPB) - nsplit[c])
        gend = (np.searchsorted(node2graph, np.arange(GPB) + c * GPB, side="right")
                - nsplit[c])
        amask = np.full((GPB, NLP), -1e30, np.float32)
        for g in range(GPB):
            amask[g, gstart[g]:gend[g]] = 0.0
        lo, hi = nsplit[c], nsplit[c + 1]
        deg = np.bincount(dst[(dst >= lo) & (dst < hi)] - lo, minlength=NLP)
        vmask = (deg[:NLP] > 0).astype(np.float32)
        cores.append(dict(src_idx=src_idx, scat=scat, selm=selm, xe_order=xe_order,
                          amask=amask, vmask=vmask))
    return dict(nsplit=nsplit, NL=NL, NLP=NLP, W=W, T_e=T_e, ET=ET, cores=cores)


# ----------------------------------------------------- walrus wait splitter --
def _split_multi_waits(nc, mybir, bass_rust, max_waits=1):
    for fn in nc.m.functions:
        for bb in fn.blocks:
            insts = bb.instructions
            i = 0
            while i < len(insts):
                ins = insts[i]
                si = ins.sync_info
                if si is not None and si.on_wait and len(si.on_wait) > max_waits:
                    waits = list(si.on_wait)
                    extra, keep = waits[:-max_waits], waits[-max_waits:]
                    si.on_wait = keep
                    for j, w in enumerate(extra):
                        nop = mybir.InstNoOp(name=f"{ins.name}-wsplit{j}")
                        nop.engine = ins.engine
                        nop.sync_info = bass_rust.SyncInfo(on_wait=[w], on_update=[])
                        insts.insert(i, nop)
                        nc.register_instruction(nop, overwrite=True)
                        i += 1
                i += 1


# ----------------------------------------------------------- device program --
def _build(plan_dims):
    import os
    _NL_ = int(os.environ.get("K_LAYERS", "6"))
    _H0_ = os.environ.get("K_H0", "1") == "1"
    _BW_ = os.environ.get("K_BUILD", "1") == "1"
    _S2S_ = os.environ.get("K_S2S", "1") == "1"
    _MSG_ = os.environ.get("K_MSG", "1") == "1"
    _GRU_ = os.environ.get("K_GRU", "1") == "1"
    import bass_rust
    from concourse import bass, mybir
    import concourse.tile as tile

    NLP, W, T_e, ET = (plan_dims["NLP"], plan_dims["W"], plan_dims["T_e"],
                       plan_dims["ET"])
    nsplit = [int(v) for v in plan_dims["nsplit"]]
    NLs = [int(v) for v in plan_dims["NL"]]
    f32, bf16, i32 = mybir.dt.float32, mybir.dt.bfloat16, mybir.dt.int32
    AF = mybir.ActivationFunctionType
    OP = mybir.AluOpType
    AX = mybir.AxisListType
    NCH = [WCH] * (NLP // WCH) + ([NLP % WCH] if NLP % WCH else [])
    ECH = [WCH] * (ET // WCH)                      # ET is a multiple of 512

    nc = bass.Bass(num_swdge_queues=4)

    def din(name, shape, dt=bf16):
        return nc.declare_dram_parameter(name, list(shape), dt, isOutput=False)

    xnp = din("xnp", [NA, NCORES * NLP])
    xnl = din("xnl", [NA, NLP])
    xeT = din("xeT", [NB, ET])
    selm = din("selm", [ET, P], bf16)
    srcb = din("srcb", [P, T_e], i32)
    scatb = din("scatb", [P, T_e], i32)
    amask = din("amask", [GPB, NLP], bf16)
    vmaskb = din("vmaskb", [P, W], f32)
    wemb = din("wemb", [NA, D])
    bnode_rep = din("bnode_rep", [P, D], f32)
    bnode_col = din("bnode_col", [D, 1], f32)
    we_l = din("we_l", [NB, EH])
    be_col = din("be_col", [EH, 1], f32)
    w1_l = din("w1_l", [EH, EH])
    b1_col = din("b1_col", [EH, 1], f32)
    t2p = din("t2p", [EH, DD])
    b2rep = din("b2rep", [P, DD])
    b2row = din("b2row", [1, DD])
    ones_row = din("ones_row", [1, P])
    wiT = din("wiT", [D, 3 * D])
    whT = din("whT", [D, 3 * D])
    gbias = din("gbias", [D, 4], f32)
    wq0 = din("wq0", [D, 4 * D]); wr0 = din("wr0", [D, 4 * D])
    wh0 = din("wh0", [D, 4 * D]); wi1 = din("wi1", [D, 4 * D])
    wh1 = din("wh1", [D, 4 * D])
    lb0 = din("lb0", [D, 4], f32)
    lb1 = din("lb1", [D, 4], f32)
    c1q = din("c1q", [D, D]); c1r = din("c1r", [D, D])
    c1b = din("c1b", [D, 1], f32)
    c2t = din("c2t", [D, OUT]); c2b = din("c2b", [OUT, 1], f32)
    idf = din("idf", [P, P], f32)
    idb = din("idb", [P, P])
    zf = din("zf", [P, D], f32)
    zb = din("zb", [P, D])
    zmt = din("zmt", [NLP + P, D], f32)
    y = nc.declare_dram_parameter("y", [OUT, GPB], f32, isOutput=True)

    with tile.TileContext(nc) as tc:
        nc.__enter_lp = nc.allow_low_precision("bf16 message path")
        nc.__enter_lp.__enter__()
        with tc.tile_pool(name="dram", bufs=1, space="DRAM") as dpool, \
             tc.tile_pool(name="const", bufs=1) as cp, \
             tc.tile_pool(name="state", bufs=1) as stp, \
             tc.tile_pool(name="sb", bufs=3) as sb, \
             tc.tile_pool(name="wstream", bufs=3) as wsp, \
             tc.tile_pool(name="prodp", bufs=2) as prp, \
             tc.tile_pool(name="ps", bufs=4, space="PSUM") as ps, \
             tc.tile_pool(name="psg", bufs=4, space="PSUM") as psg:

            w_tab = dpool.tile([ET, DD], bf16)
            m_tab = dpool.tile([NLP + P, D], f32)
            hloc = dpool.tile([NLP, D], bf16)
            hgat = dpool.tile([NCORES * NLP, D], bf16)

            ident = cp.tile([P, P], f32)
            nc.sync.dma_start(out=ident[:], in_=idf[:])
            identb = cp.tile([P, P], bf16)
            nc.sync.dma_start(out=identb[:], in_=idb[:])

            def ld(dram, shape, dt):
                nm = f"c_{dram.name}"
                t = cp.tile(list(shape), dt, name=nm, tag=nm)
                nc.sync.dma_start(out=t[:], in_=dram[:])
                return t

            wemb_s = ld(wemb, [NA, D], bf16)
            bnr_s = ld(bnode_rep, [P, D], f32)
            bnc_s = ld(bnode_col, [D, 1], f32)
            we_s = ld(we_l, [NB, EH], bf16)
            bec_s = ld(be_col, [EH, 1], f32)
            w1_s = ld(w1_l, [EH, EH], bf16)
            b1c_s = ld(b1_col, [EH, 1], f32)
            t2p_s = ld(t2p, [EH, DD], bf16)
            b2r_s = wsp.tile([P, DD], bf16, name="b2rep_s", tag="w")
            nc.sync.dma_start(out=b2r_s[:], in_=b2rep[:])
            b2row_s = ld(b2row, [1, DD], bf16)
            ones_s = ld(ones_row, [1, P], bf16)
            wiT_s = ld(wiT, [D, 3 * D], bf16)
            whT_s = ld(whT, [D, 3 * D], bf16)
            gb_s = ld(gbias, [D, 4], f32)
            srcb_s = ld(srcb, [P, T_e], i32)
            scatb_s = ld(scatb, [P, T_e], i32)
            vm_s = ld(vmaskb, [P, W], f32)
            am_s = ld(amask, [GPB, NLP], bf16)
            wq0_s = ld(wq0, [D, 4 * D], bf16); wr0_s = ld(wr0, [D, 4 * D], bf16)
            wh0_s = ld(wh0, [D, 4 * D], bf16); wi1_s = ld(wi1, [D, 4 * D], bf16)
            wh1_s = ld(wh1, [D, 4 * D], bf16)
            lb0_s = ld(lb0, [D, 4], f32)
            lb1_s = ld(lb1, [D, 4], f32)
            c1q_s = ld(c1q, [D, D], bf16); c1r_s = ld(c1r, [D, D], bf16)
            c1b_s = ld(c1b, [D, 1], f32)
            c2t_s = ld(c2t, [D, OUT], bf16)
            c2b_s = ld(c2b, [OUT, 1], f32)

            hgT = stp.tile([D, NLP], f32)
            hgTb = stp.tile([D, NLP], bf16)
            h_rm = stp.tile([P, W * D], bf16)
            mtr = stp.tile([D, NLP], bf16)

            # ---- zero m table (once) ----
            nc.scalar.dma_start(out=m_tab[:], in_=zmt[:])

            # ---- h0 full (padded coords) -> hgat ----
            NT = NCORES * NLP
            for kc in range(NT // WCH if _H0_ else 0):
                xt = sb.tile([NA, WCH], bf16, tag="xnt")
                nc.sync.dma_start(out=xt[:], in_=xnp[:, kc * WCH:(kc + 1) * WCH])
                h0ps = ps.tile([D, WCH], f32, tag="psA")
                nc.tensor.matmul(out=h0ps[:], lhsT=wemb_s[:], rhs=xt[:],
                                 start=True, stop=True)
                h0tb = sb.tile([D, WCH], bf16, tag="h0tb")
                nc.scalar.activation(out=h0tb[:], in_=h0ps[:], func=AF.Identity,
                                     bias=bnc_s[:, :1], scale=1.0)
                h0st = sb.tile([P, (WCH // P) * D], bf16, tag="h0st")
                for tt in range(WCH // P):
                    trp = psg.tile([P, D], bf16, tag="psB")
                    nc.tensor.transpose(out=trp[:],
                                        in_=h0tb[:, tt * P:(tt + 1) * P],
                                        identity=identb[:D, :D])
                    nc.scalar.copy(out=h0st[:, tt * D:(tt + 1) * D], in_=trp[:])
                nc.scalar.dma_start(
                    out=hgat[kc * WCH:(kc + 1) * WCH, :].rearrange(
                        "(t p) d -> p t d", p=P),
                    in_=h0st[:].rearrange("p (t d) -> p t d", d=D))

            # ---- h0T local -> hgT / hgTb ----
            off = 0
            for ch in NCH:
                xl = sb.tile([NA, WCH], bf16, tag="xnl")
                nc.sync.dma_start(out=xl[:, :ch], in_=xnl[:, off:off + ch])
                hps = ps.tile([D, WCH], f32, tag="psA")
                nc.tensor.matmul(out=hps[:, :ch], lhsT=wemb_s[:],
                                 rhs=xl[:, :ch], start=True, stop=True)
                nc.scalar.activation(out=hgT[:, off:off + ch], in_=hps[:, :ch],
                                     func=AF.Identity, bias=bnc_s[:, :1], scale=1.0)
                nc.vector.tensor_copy(out=hgTb[:, off:off + ch],
                                      in_=hgT[:, off:off + ch])
                off += ch

            # ---- edge MLP -> w_tab (one-time) ----
            for kc in range(len(ECH) if _BW_ else 0):
                xe_sb = sb.tile([NB, WCH], bf16, tag="xe")
                nc.sync.dma_start(out=xe_sb[:], in_=xeT[:, kc * WCH:(kc + 1) * WCH])
                he_ps = ps.tile([EH, WCH], f32, tag="psA")
                nc.tensor.matmul(out=he_ps[:], lhsT=we_s[:], rhs=xe_sb[:],
                                 start=True, stop=True)
                he_sb = sb.tile([EH, WCH], bf16, tag="hesb")
                nc.scalar.activation(out=he_sb[:], in_=he_ps[:], func=AF.Identity,
                                     bias=bec_s[:, :1], scale=1.0)
                u_ps = ps.tile([EH, WCH], f32, tag="psA")
                nc.tensor.matmul(out=u_ps[:], lhsT=w1_s[:], rhs=he_sb[:],
                                 start=True, stop=True)
                u_sb = sb.tile([EH, WCH], bf16, tag="usb")
                nc.scalar.activation(out=u_sb[:], in_=u_ps[:], func=AF.Relu,
                                     bias=b1c_s[:, :1], scale=1.0)
                for tt in range(WCH // P):
                    et = kc * (WCH // P) + tt
                    w_sb = wsp.tile([P, DD], bf16, tag="w")
                    for nb in range(DD // WCH):
                        wps = ps.tile([P, WCH], f32, tag="psA")
                        if nb % 2 == 0:
                            nc.tensor.matmul(
                                out=wps[:], lhsT=u_sb[:, tt * P:(tt + 1) * P],
                                rhs=t2p_s[:, nb * WCH:(nb + 1) * WCH],
                                start=True, stop=False)
                            nc.tensor.matmul(
                                out=wps[:], lhsT=ones_s[:, :P],
                                rhs=b2row_s[:, nb * WCH:(nb + 1) * WCH],
                                start=False, stop=True)
                            nc.scalar.copy(out=w_sb[:, nb * WCH:(nb + 1) * WCH],
                                           in_=wps[:])
                        else:
                            nc.tensor.matmul(
                                out=wps[:], lhsT=u_sb[:, tt * P:(tt + 1) * P],
                                rhs=t2p_s[:, nb * WCH:(nb + 1) * WCH],
                                start=True, stop=True)
                            nc.vector.tensor_tensor(
                                out=w_sb[:, nb * WCH:(nb + 1) * WCH], in0=wps[:],
                                in1=b2r_s[:, nb * WCH:(nb + 1) * WCH], op=OP.add)
                    nc.sync.dma_start(out=w_tab[et * P:(et + 1) * P, :], in_=w_sb[:])

            # ================= 6 MPNN layers =================
            for layer in range(min(L, _NL_)):
                for t in range(T_e if _MSG_ else 0):
                    hs4 = sb.tile([P, D], bf16, tag="hsrc4")
                    nc.gpsimd.indirect_dma_start(
                        out=hs4[:], out_offset=None, in_=hgat[:],
                        in_offset=bass.IndirectOffsetOnAxis(
                            ap=srcb_s[:, t:t + 1], axis=0))
                    hs = hs4[:, :]
                    wt = wsp.tile([P, DD], bf16, tag="w")
                    nc.sync.dma_start(out=wt[:], in_=w_tab[t * P:(t + 1) * P, :])
                    prod = prp.tile([P, DD], bf16, tag="prod")
                    nc.vector.tensor_tensor(
                        out=prod[:].rearrange("p (o i) -> p o i", o=D),
                        in0=wt[:].rearrange("p (o i) -> p o i", o=D),
                        in1=hs.unsqueeze(1).broadcast_to([P, D, D]),
                        op=OP.mult)
                    pv = prod[:].rearrange("p (o i) -> p o i", o=D)
                    t32 = prp.tile([P, D * 32], bf16, tag="t32")
                    nc.vector.tensor_tensor(
                        out=t32[:].rearrange("p (o i) -> p o i", o=D),
                        in0=pv[:, :, 0:32], in1=pv[:, :, 32:64], op=OP.add)
                    tv = t32[:].rearrange("p (o i) -> p o i", o=D)
                    t16 = prp.tile([P, D * 16], bf16, tag="t16")
                    nc.vector.tensor_tensor(
                        out=t16[:].rearrange("p (o i) -> p o i", o=D),
                        in0=tv[:, :, 0:16], in1=tv[:, :, 16:32], op=OP.add)
                    tv = t16[:].rearrange("p (o i) -> p o i", o=D)
                    t8 = prp.tile([P, D * 8], bf16, tag="t8")
                    nc.vector.tensor_tensor(
                        out=t8[:].rearrange("p (o i) -> p o i", o=D),
                        in0=tv[:, :, 0:8], in1=tv[:, :, 8:16], op=OP.add)
                    m_e = sb.tile([P, D], bf16, tag="me")
                    nc.vector.tensor_reduce(
                        out=m_e[:], in_=t8[:].rearrange("p (o i) -> p o i", o=D),
                        axis=AX.X, op=OP.add)
                    selt = sb.tile([P, P], bf16, tag="sel")
                    nc.sync.dma_start(out=selt[:], in_=selm[t * P:(t + 1) * P, :])
                    rows_ps = psg.tile([P, D], f32, tag="psB")
                    nc.tensor.matmul(out=rows_ps[:], lhsT=selt[:], rhs=m_e[:],
                                     start=True, stop=True)
                    rows = sb.tile([P, D], f32, tag="rows")
                    nc.scalar.copy(out=rows[:], in_=rows_ps[:])
                    nc.gpsimd.indirect_dma_start(
                        out=m_tab[:], out_offset=bass.IndirectOffsetOnAxis(
                            ap=scatb_s[:, t:t + 1], axis=0),
                        in_=rows[:], in_offset=None)

                # phase A: m windows -> relu/mask -> transposed mtr (resident)
                for w in range(W if _GRU_ else 0):
                    mw = sb.tile([P, D], f32, tag="mw")
                    nc.sync.dma_start(out=mw[:], in_=m_tab[w * P:(w + 1) * P, :])
                    mwm = sb.tile([P, D], f32, tag="mwm")
                    nc.scalar.activation(out=mwm[:], in_=mw[:], func=AF.Copy,
                                         scale=vm_s[:, w:w + 1])
                    mt_ps = psg.tile([D, P], f32, tag="psB")
                    nc.tensor.transpose(out=mt_ps[:], in_=mwm[:], identity=ident[:])
                    nc.scalar.activation(out=mtr[:, w * P:(w + 1) * P],
                                         in_=mt_ps[:], func=AF.Relu)
                # phase B: batched GRU over 512-wide chunks
                off = 0
                for ch in (NCH if _GRU_ else []):
                    sl = slice(off, off + ch)
                    mch = mtr[:, sl]
                    hch = hgTb[:, sl]
                    ps_r = ps.tile([D, WCH], f32, tag="psA")
                    nc.tensor.matmul(out=ps_r[:, :ch], lhsT=wiT_s[:, 0:D],
                                     rhs=mch, start=True, stop=False)
                    nc.tensor.matmul(out=ps_r[:, :ch], lhsT=whT_s[:, 0:D],
                                     rhs=hch, start=False, stop=True)
                    r_t = sb.tile([D, WCH], f32, tag="r_t", bufs=2)
                    nc.scalar.activation(out=r_t[:, :ch], in_=ps_r[:, :ch],
                                         func=AF.Sigmoid, bias=gb_s[:, 0:1],
                                         scale=1.0)
                    ps_z = ps.tile([D, WCH], f32, tag="psA")
                    nc.tensor.matmul(out=ps_z[:, :ch], lhsT=wiT_s[:, D:2 * D],
                                     rhs=mch, start=True, stop=False)
                    nc.tensor.matmul(out=ps_z[:, :ch], lhsT=whT_s[:, D:2 * D],
                                     rhs=hch, start=False, stop=True)
                    z_t = sb.tile([D, WCH], f32, tag="z_t", bufs=2)
                    nc.scalar.activation(out=z_t[:, :ch], in_=ps_z[:, :ch],
                                         func=AF.Sigmoid, bias=gb_s[:, 1:2],
                                         scale=1.0)
                    ps_xn = ps.tile([D, WCH], f32, tag="psA")
                    nc.tensor.matmul(out=ps_xn[:, :ch], lhsT=wiT_s[:, 2 * D:3 * D],
                                     rhs=mch, start=True, stop=True)
                    gxn = sb.tile([D, WCH], f32, tag="gxn", bufs=2)
                    nc.scalar.activation(out=gxn[:, :ch], in_=ps_xn[:, :ch],
                                         func=AF.Identity, bias=gb_s[:, 2:3],
                                         scale=1.0)
                    ps_hn = ps.tile([D, WCH], f32, tag="psA")
                    nc.tensor.matmul(out=ps_hn[:, :ch], lhsT=whT_s[:, 2 * D:3 * D],
                                     rhs=hch, start=True, stop=True)
                    ghn = sb.tile([D, WCH], f32, tag="ghn", bufs=2)
                    nc.scalar.activation(out=ghn[:, :ch], in_=ps_hn[:, :ch],
                                         func=AF.Identity, bias=gb_s[:, 3:4],
                                         scale=1.0)
                    t1 = sb.tile([D, WCH], f32, tag="t1", bufs=2)
                    nc.vector.tensor_tensor(out=t1[:, :ch], in0=r_t[:, :ch],
                                            in1=ghn[:, :ch], op=OP.mult)
                    nc.vector.tensor_tensor(out=t1[:, :ch], in0=t1[:, :ch],
                                            in1=gxn[:, :ch], op=OP.add)
                    n_t = sb.tile([D, WCH], f32, tag="n_t", bufs=2)
                    nc.scalar.activation(out=n_t[:, :ch], in_=t1[:, :ch],
                                         func=AF.Tanh)
                    hgch = hgT[:, sl]
                    nc.vector.tensor_tensor(out=t1[:, :ch], in0=hgch,
                                            in1=n_t[:, :ch], op=OP.subtract)
                    nc.vector.tensor_tensor(out=t1[:, :ch], in0=z_t[:, :ch],
                                            in1=t1[:, :ch], op=OP.mult)
                    nc.vector.tensor_tensor(out=hgch, in0=t1[:, :ch],
                                            in1=n_t[:, :ch], op=OP.add)
                    nc.vector.tensor_copy(out=hgTb[:, sl], in_=hgch)
                    off += ch
                # phase C: transposed h back to row-major for sharing/readout
                for w in range(W if _GRU_ else 0):
                    hgb_w = hgTb[:, w * P:(w + 1) * P]
                    hr_ps = psg.tile([P, D], bf16, tag="psB")
                    nc.tensor.transpose(out=hr_ps[:], in_=hgb_w,
                                        identity=identb[:D, :D])
                    if layer < L - 1:
                        hr_sb = sb.tile([P, D], bf16, tag="hr_sb")
                        nc.scalar.copy(out=hr_sb[:], in_=hr_ps[:])
                        nc.sync.dma_start(out=hloc[w * P:(w + 1) * P, :],
                                          in_=hr_sb[:])
                    else:
                        nc.scalar.copy(out=h_rm[:, w * D:(w + 1) * D], in_=hr_ps[:])

                if layer < L - 1:
                    nc.gpsimd.collective_compute(
                        "AllGather", OP.bypass,
                        replica_groups=[list(range(NCORES))],
                        ins=[hloc[:].opt()], outs=[hgat[:].opt()])

            # ================= Set2Set =================
            qTb = stp.tile([D, GPB], bf16)
            rTb = stp.tile([D, GPB], bf16)
            hT0 = stp.tile([D, GPB], f32)
            cT0 = stp.tile([D, GPB], f32)
            hT1 = stp.tile([D, GPB], f32)
            cT1 = stp.tile([D, GPB], f32)
            h0b = stp.tile([D, GPB], bf16)
            h1b = stp.tile([D, GPB], bf16)
            e_sb = stp.tile([GPB, NLP], f32)
            al_b = stp.tile([GPB, NLP], bf16)
            for tl in (qTb, rTb, h0b, h1b):
                nc.scalar.dma_start(out=tl[:], in_=zb[:D, :GPB])
            for tl in (hT0, cT0, hT1, cT1):
                nc.scalar.dma_start(out=tl[:], in_=zf[:D, :GPB])

            def lstm_layer(wx_parts, wh_s, h_b, hT, cT, lb_s, out_b):
                gates = []
                for g in range(4):
                    pst = psg.tile([D, GPB], f32, tag="psB")
                    first = True
                    for (wt_s, rhs_t) in wx_parts:
                        nc.tensor.matmul(out=pst[:],
                                         lhsT=wt_s[:, g * D:(g + 1) * D],
                                         rhs=rhs_t[:], start=first, stop=False)
                        first = False
                    nc.tensor.matmul(out=pst[:], lhsT=wh_s[:, g * D:(g + 1) * D],
                                     rhs=h_b[:], start=False, stop=True)
                    fn = AF.Tanh if g == 2 else AF.Sigmoid
                    gt = sb.tile([D, GPB], f32, tag=f"lstm_g{g}")
                    nc.scalar.activation(out=gt[:], in_=pst[:], func=fn,
                                         bias=lb_s[:, g:g + 1], scale=1.0)
                    gates.append(gt)
                ig, fg, gg, og = gates
                fc = sb.tile([D, GPB], f32, tag="fc")
                nc.vector.tensor_tensor(out=fc[:], in0=fg[:], in1=cT[:], op=OP.mult)
                igg = sb.tile([D, GPB], f32, tag="igg")
                nc.vector.tensor_tensor(out=igg[:], in0=ig[:], in1=gg[:], op=OP.mult)
                nc.vector.tensor_tensor(out=cT[:], in0=fc[:], in1=igg[:], op=OP.add)
                tc_ = sb.tile([D, GPB], f32, tag="tc_")
                nc.scalar.activation(out=tc_[:], in_=cT[:], func=AF.Tanh)
                nc.vector.tensor_tensor(out=hT[:], in0=og[:], in1=tc_[:], op=OP.mult)
                nc.vector.tensor_copy(out=out_b[:], in_=hT[:])

            for it in range(ITERS if _S2S_ else 0):
                lstm_layer([(wq0_s, qTb), (wr0_s, rTb)], wh0_s, h0b, hT0, cT0,
                           lb0_s, h0b)
                lstm_layer([(wi1_s, h0b)], wh1_s, h1b, hT1, cT1, lb1_s, h1b)
                nc.vector.tensor_copy(out=qTb[:], in_=hT1[:])

                off = 0
                for ch in NCH:
                    eps = ps.tile([GPB, WCH], f32, tag="psA")
                    nc.tensor.matmul(out=eps[:, :ch], lhsT=qTb[:],
                                     rhs=hgTb[:, off:off + ch], start=True,
                                     stop=True)
                    nc.vector.tensor_tensor(out=e_sb[:, off:off + ch],
                                            in0=eps[:, :ch],
                                            in1=am_s[:, off:off + ch], op=OP.add)
                    off += ch
                nmax = sb.tile([GPB, 1], f32, tag="nmax")
                nc.vector.tensor_reduce(out=nmax[:], in_=e_sb[:], axis=AX.X,
                                        op=OP.max, negate=True)
                ssum = sb.tile([GPB, 1], f32, tag="ssum")
                nc.scalar.activation(out=e_sb[:], in_=e_sb[:], func=AF.Exp,
                                     bias=nmax[:, :1], scale=1.0,
                                     accum_out=ssum[:])
                rsum = sb.tile([GPB, 1], f32, tag="rsum")
                nc.vector.reciprocal(out=rsum[:], in_=ssum[:])
                nc.vector.tensor_scalar_mul(al_b[:], e_sb[:], rsum[:, :1])

                ro_ps = psg.tile([D, GPB], f32, tag="psB")
                for w in range(W):
                    at_ps = psg.tile([P, GPB], bf16, tag="psB")
                    nc.tensor.transpose(out=at_ps[:],
                                        in_=al_b[:, w * P:(w + 1) * P],
                                        identity=identb[:GPB, :GPB])
                    at_b = sb.tile([P, GPB], bf16, tag="at_b")
                    nc.scalar.copy(out=at_b[:], in_=at_ps[:])
                    nc.tensor.matmul(
                        out=ro_ps[:], lhsT=h_rm[:, w * D:(w + 1) * D],
                        rhs=at_b[:], start=(w == 0), stop=(w == W - 1))
                nc.vector.tensor_copy(out=rTb[:], in_=ro_ps[:])

            # ================= classifier =================
            ps1 = psg.tile([D, GPB], f32, tag="psB")
            nc.tensor.matmul(out=ps1[:], lhsT=c1q_s[:], rhs=qTb[:],
                             start=True, stop=False)
            nc.tensor.matmul(out=ps1[:], lhsT=c1r_s[:], rhs=rTb[:],
                             start=False, stop=True)
            z1b = sb.tile([D, GPB], bf16, tag="z1b")
            nc.scalar.activation(out=z1b[:], in_=ps1[:], func=AF.Relu,
                                 bias=c1b_s[:, :1], scale=1.0)
            ps2 = psg.tile([OUT, GPB], f32, tag="psB")
            nc.tensor.matmul(out=ps2[:], lhsT=c2t_s[:], rhs=z1b[:],
                             start=True, stop=True)
            yout = sb.tile([OUT, GPB], f32, tag="yout")
            nc.scalar.activation(out=yout[:], in_=ps2[:], func=AF.Identity,
                                 bias=c2b_s[:, :1], scale=1.0)
            nc.sync.dma_start(out=y[:], in_=yout[:])
        nc.__enter_lp.__exit__(None, None, None)

    _split_multi_waits(nc, mybir, bass_rust)
    return nc


# ------------------------------------------------------------------- driver --
def kernel(x_node, x_edge, params, src, dst, node2graph):
    from concourse.bass_utils import run_bass_kernel_spmd

    x_node = np.asarray(x_node, np.float32)
    x_edge = np.asarray(x_edge, np.float32)
    src = np.asarray(src, np.int32)
    dst = np.asarray(dst, np.int32)
    node2graph = np.asarray(node2graph, np.int32)
    p = {k: np.asarray(v, np.float32) for k, v in params.items()}

    plan = _plan(src, dst, node2graph)
    NLP, W, T_e, ET = plan["NLP"], plan["W"], plan["T_e"], plan["ET"]
    nsplit = plan["nsplit"]

    key = (NLP, T_e, tuple(int(v) for v in nsplit))
    if key not in _CACHE:
        _CACHE[key] = _build(plan)
    nc = _CACHE[key]

    T2p = p["en_w2"].reshape(D, D, EH).transpose(2, 1, 0).reshape(EH, DD)
    b2p = p["en_b2"].reshape(D, D).T.reshape(1, DD)
    gb = np.stack([
        p["gru_bi"][:D] + p["gru_bh"][:D],
        p["gru_bi"][D:2 * D] + p["gru_bh"][D:2 * D],
        p["gru_bi"][2 * D:],
        p["gru_bh"][2 * D:],
    ], axis=1).astype(np.float32)
    lb0 = (p["lstm_bih0"] + p["lstm_bhh0"]).reshape(4, D).T.copy().astype(np.float32)
    lb1 = (p["lstm_bih1"] + p["lstm_bhh1"]).reshape(4, D).T.copy().astype(np.float32)

    NT = NCORES * NLP
    xnp = np.zeros((NA, NT), np.float32)
    for c in range(NCORES):
        lo, hi = int(nsplit[c]), int(nsplit[c + 1])
        xnp[:, c * NLP:c * NLP + hi - lo] = x_node[lo:hi].T
    b2rep_np = np.broadcast_to(b2p, (P, DD)).copy()
    rep = {
        "xnp": xnp.astype(BF),
        "wemb": np.ascontiguousarray(p["node_emb_w"].T).astype(BF),
        "bnode_rep": np.broadcast_to(p["node_emb_b"], (P, D)).copy().astype(np.float32),
        "bnode_col": p["node_emb_b"].reshape(D, 1).astype(np.float32),
        "we_l": np.ascontiguousarray(p["edge_emb_w"].T).astype(BF),
        "be_col": p["edge_emb_b"].reshape(EH, 1).astype(np.float32),
        "w1_l": np.ascontiguousarray(p["en_w1"].T).astype(BF),
        "b1_col": p["en_b1"].reshape(EH, 1).astype(np.float32),
        "t2p": np.ascontiguousarray(T2p).astype(BF),
        "b2rep": b2rep_np.astype(BF),
        "b2row": np.ascontiguousarray(b2p).astype(BF),
        "ones_row": np.ones((1, P), BF),
        "wiT": np.ascontiguousarray(p["gru_wi"].T).astype(BF),
        "whT": np.ascontiguousarray(p["gru_wh"].T).astype(BF),
        "gbias": gb,
        "wq0": np.ascontiguousarray(p["lstm_wih0"][:, :D].T).astype(BF),
        "wr0": np.ascontiguousarray(p["lstm_wih0"][:, D:].T).astype(BF),
        "wh0": np.ascontiguousarray(p["lstm_whh0"].T).astype(BF),
        "wi1": np.ascontiguousarray(p["lstm_wih1"].T).astype(BF),
        "wh1": np.ascontiguousarray(p["lstm_whh1"].T).astype(BF),
        "lb0": lb0, "lb1": lb1,
        "c1q": np.ascontiguousarray(p["c1_w"][:, :D].T).astype(BF),
        "c1r": np.ascontiguousarray(p["c1_w"][:, D:].T).astype(BF),
        "c1b": p["c1_b"].reshape(D, 1).astype(np.float32),
        "c2t": np.ascontiguousarray(p["c2_w"].T).astype(BF),
        "c2b": p["c2_b"].reshape(OUT, 1).astype(np.float32),
        "idf": np.eye(P, dtype=np.float32),
        "idb": np.eye(P, dtype=np.float32).astype(BF),
        "zf": np.zeros((P, D), np.float32),
        "zb": np.zeros((P, D), BF),
    }

    in_maps = []
    for c in range(NCORES):
        pc = plan["cores"][c]
        lo, hi = int(nsplit[c]), int(nsplit[c + 1])
        xnl = np.zeros((NA, NLP), np.float32)
        xnl[:, :hi - lo] = x_node[lo:hi].T
        m = dict(rep)
        m["zmt"] = np.zeros((NLP + P, D), np.float32)
        m["xnl"] = xnl.astype(BF)
        m["xeT"] = np.ascontiguousarray(x_edge[pc["xe_order"]].T).astype(BF)
        m["selm"] = pc["selm"].astype(BF)
        m["srcb"] = np.ascontiguousarray(pc["src_idx"].reshape(T_e, P).T)
        m["scatb"] = np.ascontiguousarray(pc["scat"].reshape(T_e, P).T)
        m["amask"] = pc["amask"].astype(BF)
        m["vmaskb"] = np.ascontiguousarray(
            pc["vmask"].reshape(W, P).T).astype(np.float32)
        in_maps.append(m)

    res = run_bass_kernel_spmd(nc, in_maps, list(range(NCORES)))
    out = np.concatenate([res.results[c]["y"].T for c in range(NCORES)], axis=0)
    return out.astype(np.float32)


# revision 18
# speedup vs baseline: 1.4632x; 1.1576x over previous
"""Trainium2 Bass kernel for the MPNN discriminator (NNConv+GRU x6, Set2Set, MLP).

Self-contained: takes FULL inputs, shards across 8 NeuronCores internally,
returns the FULL [512, 2] output.

Strategy (8 cores, SPMD single program, per-core data):
- Graphs split 64-per-core; node ranges follow graph boundaries (node2graph is
  sorted). Edges assigned to the core owning dst, sorted by dst, tiled into
  128-edge tiles with no dst spanning two tiles (host pads with dummy edges
  whose src points at an always-zero h row).
- Edge MLP runs once on device; per-edge weight matrices w_e (en_b2 baked in)
  are materialized to DRAM as bf16 in [e, o*64+i] layout.
- Per layer: indirect-gather h[src] (bf16) -> DVE broadcast-multiply against
  streamed w rows -> grouped reduce over i -> per-edge messages; a host-built
  selection matmul (inv_cnt folded) sums duplicate-dst rows; rows are
  indirect-scattered to a local m table; dense 128-node windows then run
  relu+mask + GRU with PE matmuls in transposed layout; AllGather shares h.
- Set2Set runs fully local (graph-aligned shard) in transposed layout with an
  additive -1e30 mask for the segment softmax; classifier emits [2, 64] per
  core, host concatenates.
"""
import sys
sys.path.insert(0, "/opt/trn_rl_repo")
import numpy as np
import ml_dtypes

N, E, B = 25600, 51200, 512
D, NA, NB, EH = 64, 40, 10, 128
L, ITERS, OUT = 6, 6, 2
NCORES = 8
GPB = B // NCORES
P = 128
DD = D * D
WCH = 512            # psum free-dim chunk
BF = ml_dtypes.bfloat16

_CACHE = {}


# ---------------------------------------------------------------- host plan --
def _plan(src, dst, node2graph):
    nsplit = np.searchsorted(node2graph, np.arange(NCORES + 1) * GPB).astype(np.int64)
    NL = nsplit[1:] - nsplit[:-1]
    NLP = int(np.ceil(NL.max() / P) * P)
    W = NLP // P

    cnt = np.maximum(np.bincount(dst, minlength=N).astype(np.float32), 1.0)
    inv_cnt = (1.0 / cnt).astype(np.float32)
    owner = np.searchsorted(nsplit, dst, side="right") - 1

    per_core = []
    for c in range(NCORES):
        sel = np.where(owner == c)[0]
        order = np.argsort(dst[sel], kind="stable")
        eids = sel[order]
        dl = dst[eids] - nsplit[c]
        tiles, cur = [], []
        i, n = 0, len(eids)
        while i < n:
            j = i
            while j < n and dl[j] == dl[i]:
                j += 1
            if len(cur) + (j - i) > P:
                cur.extend([-1] * (P - len(cur)))
                tiles.append(cur); cur = []
            cur.extend(range(i, j))
            i = j
        if cur:
            cur.extend([-1] * (P - len(cur)))
            tiles.append(cur)
        per_core.append((eids, dl, tiles))

    T_e = max(len(t) for _, _, t in per_core)
    T_e = int(np.ceil(T_e / 4) * 4)          # ET multiple of 512 for chunking
    ET = T_e * P

    cores = []
    for c in range(NCORES):
        eids, dl, tiles = per_core[c]
        while len(tiles) < T_e:
            tiles.append([-1] * P)
        pos = np.array(tiles, dtype=np.int64).reshape(-1)
        valid = pos >= 0
        posc = np.clip(pos, 0, None)
        e_glob = np.where(valid, eids[posc], 0)
        # padded h-table coords: row = owner*NLP + (src - nsplit[owner]);
        # dummy rows contribute zero via the selection matrix, so any
        # finite row works -- use 0.
        sown = np.searchsorted(nsplit, src[e_glob], side="right") - 1
        spad = sown * NLP + (src[e_glob] - nsplit[sown])
        src_idx = np.where(valid, spad, 0).astype(np.int32)
        dst_loc = np.where(valid, dl[posc], 0)
        slot = np.arange(ET) % P
        scat = np.where(valid, dst_loc, NLP + slot).astype(np.int32)
        # dense-row position of each local node's message row (first slot whose
        # dst matches); nodes with no incoming edges -> zero block at ET
        mpos = np.full(NLP, ET, np.int64)
        vpos = np.where(valid)[0]
        # reversed so the FIRST occurrence wins
        mpos[dst_loc[vpos][::-1]] = vpos[::-1]
        selm = np.zeros((ET, P), np.float32)
        dmat = scat.reshape(T_e, P)
        vmat = valid.reshape(T_e, P)
        for t in range(T_e):
            eq = dmat[t][:, None] == dmat[t][None, :]
            gd = np.where(vmat[t], dmat[t] + nsplit[c], 0)
            ic = np.where(vmat[t], inv_cnt[gd], 0.0)
            selm[t * P:(t + 1) * P] = eq * ic[None, :]
        xe_order = np.where(valid, e_glob, 0).astype(np.int64)

        gstart = (np.searchsorted(node2graph, np.arange(GPB) + c * GPB) - nsplit[c])
        gend = (np.searchsorted(node2graph, np.arange(GPB) + c * GPB, side="right")
                - nsplit[c])
        amask = np.full((GPB, NLP), -1e30, np.float32)
        for g in range(GPB):
            amask[g, gstart[g]:gend[g]] = 0.0
        lo, hi = nsplit[c], nsplit[c + 1]
        deg = np.bincount(dst[(dst >= lo) & (dst < hi)] - lo, minlength=NLP)
        vmask = (deg[:NLP] > 0).astype(np.float32)
        cores.append(dict(src_idx=src_idx, scat=scat, selm=selm, xe_order=xe_order,
                          amask=amask, vmask=vmask, mpos=mpos.astype(np.int32)))
    return dict(nsplit=nsplit, NL=NL, NLP=NLP, W=W, T_e=T_e, ET=ET, cores=cores)


# ----------------------------------------------------- walrus wait splitter --
def _split_multi_waits(nc, mybir, bass_rust, max_waits=1):
    for fn in nc.m.functions:
        for bb in fn.blocks:
            insts = bb.instructions
            i = 0
            while i < len(insts):
                ins = insts[i]
                si = ins.sync_info
                if si is not None and si.on_wait and len(si.on_wait) > max_waits:
                    waits = list(si.on_wait)
                    extra, keep = waits[:-max_waits], waits[-max_waits:]
                    si.on_wait = keep
                    for j, w in enumerate(extra):
                        nop = mybir.InstNoOp(name=f"{ins.name}-wsplit{j}")
                        nop.engine = ins.engine
                        nop.sync_info = bass_rust.SyncInfo(on_wait=[w], on_update=[])
                        insts.insert(i, nop)
                        nc.register_instruction(nop, overwrite=True)
                        i += 1
                i += 1


# ----------------------------------------------------------- device program --
def _build(plan_dims):
    import os
    _NL_ = int(os.environ.get("K_LAYERS", "6"))
    _H0_ = os.environ.get("K_H0", "1") == "1"
    _BW_ = os.environ.get("K_BUILD", "1") == "1"
    _S2S_ = os.environ.get("K_S2S", "1") == "1"
    _MSG_ = os.environ.get("K_MSG", "1") == "1"
    _GRU_ = os.environ.get("K_GRU", "1") == "1"
    import bass_rust
    from concourse import bass, mybir
    import concourse.tile as tile

    NLP, W, T_e, ET = (plan_dims["NLP"], plan_dims["W"], plan_dims["T_e"],
                       plan_dims["ET"])
    nsplit = [int(v) for v in plan_dims["nsplit"]]
    NLs = [int(v) for v in plan_dims["NL"]]
    f32, bf16, i32 = mybir.dt.float32, mybir.dt.bfloat16, mybir.dt.int32
    AF = mybir.ActivationFunctionType
    OP = mybir.AluOpType
    AX = mybir.AxisListType
    NCH = [WCH] * (NLP // WCH) + ([NLP % WCH] if NLP % WCH else [])
    ECH = [WCH] * (ET // WCH)                      # ET is a multiple of 512

    nc = bass.Bass(num_swdge_queues=4)

    def din(name, shape, dt=bf16):
        return nc.declare_dram_parameter(name, list(shape), dt, isOutput=False)

    xnp = din("xnp", [NA, NCORES * NLP])
    xnl = din("xnl", [NA, NLP])
    xeT = din("xeT", [NB, ET])
    selm = din("selm", [ET, P], bf16)
    srcb = din("srcb", [P, T_e], i32)
    scatb = din("scatb", [P, T_e], i32)
    amask = din("amask", [GPB, NLP], bf16)
    vmaskb = din("vmaskb", [P, W], f32)
    wemb = din("wemb", [NA, D])
    bnode_rep = din("bnode_rep", [P, D], f32)
    bnode_col = din("bnode_col", [D, 1], f32)
    we_l = din("we_l", [NB, EH])
    be_col = din("be_col", [EH, 1], f32)
    w1_l = din("w1_l", [EH, EH])
    b1_col = din("b1_col", [EH, 1], f32)
    t2p = din("t2p", [EH, DD])
    b2rep = din("b2rep", [P, DD])
    b2row = din("b2row", [1, DD])
    ones_row = din("ones_row", [1, P])
    wiT = din("wiT", [D, 3 * D])
    whT = din("whT", [D, 3 * D])
    gbias = din("gbias", [D, 4], f32)
    wq0 = din("wq0", [D, 4 * D]); wr0 = din("wr0", [D, 4 * D])
    wh0 = din("wh0", [D, 4 * D]); wi1 = din("wi1", [D, 4 * D])
    wh1 = din("wh1", [D, 4 * D])
    lb0 = din("lb0", [D, 4], f32)
    lb1 = din("lb1", [D, 4], f32)
    c1q = din("c1q", [D, D]); c1r = din("c1r", [D, D])
    c1b = din("c1b", [D, 1], f32)
    c2t = din("c2t", [D, OUT]); c2b = din("c2b", [OUT, 1], f32)
    idf = din("idf", [P, P], f32)
    idb = din("idb", [P, P])
    zf = din("zf", [P, D], f32)
    zb = din("zb", [P, D])
    zmt = din("zmt", [P, D], f32)
    mwoff = din("mwoff", [P, W], i32)
    y = nc.declare_dram_parameter("y", [OUT, GPB], f32, isOutput=True)

    with tile.TileContext(nc) as tc:
        nc.__enter_lp = nc.allow_low_precision("bf16 message path")
        nc.__enter_lp.__enter__()
        with tc.tile_pool(name="dram", bufs=1, space="DRAM") as dpool, \
             tc.tile_pool(name="const", bufs=1) as cp, \
             tc.tile_pool(name="state", bufs=1) as stp, \
             tc.tile_pool(name="sb", bufs=3) as sb, \
             tc.tile_pool(name="wstream", bufs=3) as wsp, \
             tc.tile_pool(name="prodp", bufs=2) as prp, \
             tc.tile_pool(name="ps", bufs=4, space="PSUM") as ps, \
             tc.tile_pool(name="psg", bufs=4, space="PSUM") as psg:

            w_tabs = [dpool.tile([ET // 4, DD], bf16, name=f"w_tab{q}",
                                 tag=f"w_tab{q}") for q in range(4)]
            m_tab = dpool.tile([ET + P, D], f32)
            hloc = dpool.tile([NLP, D], bf16)
            hgat = dpool.tile([NCORES * NLP, D], bf16)

            ident = cp.tile([P, P], f32)
            nc.sync.dma_start(out=ident[:], in_=idf[:])
            identb = cp.tile([P, P], bf16)
            nc.sync.dma_start(out=identb[:], in_=idb[:])

            def ld(dram, shape, dt):
                nm = f"c_{dram.name}"
                t = cp.tile(list(shape), dt, name=nm, tag=nm)
                nc.sync.dma_start(out=t[:], in_=dram[:])
                return t

            wemb_s = ld(wemb, [NA, D], bf16)
            bnr_s = ld(bnode_rep, [P, D], f32)
            bnc_s = ld(bnode_col, [D, 1], f32)
            we_s = ld(we_l, [NB, EH], bf16)
            bec_s = ld(be_col, [EH, 1], f32)
            w1_s = ld(w1_l, [EH, EH], bf16)
            b1c_s = ld(b1_col, [EH, 1], f32)
            t2p_s = ld(t2p, [EH, DD], bf16)
            b2r_s = wsp.tile([P, DD], bf16, name="b2rep_s", tag="w")
            nc.sync.dma_start(out=b2r_s[:], in_=b2rep[:])
            b2row_s = ld(b2row, [1, DD], bf16)
            ones_s = ld(ones_row, [1, P], bf16)
            wiT_s = ld(wiT, [D, 3 * D], bf16)
            whT_s = ld(whT, [D, 3 * D], bf16)
            gb_s = ld(gbias, [D, 4], f32)
            srcb_s = ld(srcb, [P, T_e], i32)
            scatb_s = ld(scatb, [P, T_e], i32)
            vm_s = ld(vmaskb, [P, W], f32)
            mwo_s = ld(mwoff, [P, W], i32)
            am_s = ld(amask, [GPB, NLP], bf16)
            wq0_s = ld(wq0, [D, 4 * D], bf16); wr0_s = ld(wr0, [D, 4 * D], bf16)
            wh0_s = ld(wh0, [D, 4 * D], bf16); wi1_s = ld(wi1, [D, 4 * D], bf16)
            wh1_s = ld(wh1, [D, 4 * D], bf16)
            lb0_s = ld(lb0, [D, 4], f32)
            lb1_s = ld(lb1, [D, 4], f32)
            c1q_s = ld(c1q, [D, D], bf16); c1r_s = ld(c1r, [D, D], bf16)
            c1b_s = ld(c1b, [D, 1], f32)
            c2t_s = ld(c2t, [D, OUT], bf16)
            c2b_s = ld(c2b, [OUT, 1], f32)

            hgT = stp.tile([D, NLP], f32)
            hgTb = stp.tile([D, NLP], bf16)
            h_rm = stp.tile([P, W * D], bf16)
            mtr = stp.tile([D, NLP], bf16)

            # ---- zero the no-edge row block of the dense m table (once) ----
            nc.scalar.dma_start(out=m_tab[ET:ET + P, :], in_=zmt[:])

            # ---- h0 full (padded coords) -> hgat ----
            NT = NCORES * NLP
            for kc in range(NT // WCH if _H0_ else 0):
                xt = sb.tile([NA, WCH], bf16, tag="xnt")
                nc.sync.dma_start(out=xt[:], in_=xnp[:, kc * WCH:(kc + 1) * WCH])
                h0ps = ps.tile([D, WCH], f32, tag="psA")
                nc.tensor.matmul(out=h0ps[:], lhsT=wemb_s[:], rhs=xt[:],
                                 start=True, stop=True)
                h0tb = sb.tile([D, WCH], bf16, tag="h0tb")
                nc.scalar.activation(out=h0tb[:], in_=h0ps[:], func=AF.Identity,
                                     bias=bnc_s[:, :1], scale=1.0)
                h0st = sb.tile([P, (WCH // P) * D], bf16, tag="h0st")
                for tt in range(WCH // P):
                    trp = psg.tile([P, D], bf16, tag="psB")
                    nc.tensor.transpose(out=trp[:],
                                        in_=h0tb[:, tt * P:(tt + 1) * P],
                                        identity=identb[:D, :D])
                    nc.scalar.copy(out=h0st[:, tt * D:(tt + 1) * D], in_=trp[:])
                nc.scalar.dma_start(
                    out=hgat[kc * WCH:(kc + 1) * WCH, :].rearrange(
                        "(t p) d -> p t d", p=P),
                    in_=h0st[:].rearrange("p (t d) -> p t d", d=D))

            # ---- h0T local -> hgT / hgTb ----
            off = 0
            for ch in NCH:
                xl = sb.tile([NA, WCH], bf16, tag="xnl")
                nc.sync.dma_start(out=xl[:, :ch], in_=xnl[:, off:off + ch])
                hps = ps.tile([D, WCH], f32, tag="psA")
                nc.tensor.matmul(out=hps[:, :ch], lhsT=wemb_s[:],
                                 rhs=xl[:, :ch], start=True, stop=True)
                nc.scalar.activation(out=hgT[:, off:off + ch], in_=hps[:, :ch],
                                     func=AF.Identity, bias=bnc_s[:, :1], scale=1.0)
                nc.vector.tensor_copy(out=hgTb[:, off:off + ch],
                                      in_=hgT[:, off:off + ch])
                off += ch

            # ---- edge MLP -> w_tab (one-time) ----
            for kc in range(len(ECH) if _BW_ else 0):
                xe_sb = sb.tile([NB, WCH], bf16, tag="xe")
                nc.sync.dma_start(out=xe_sb[:], in_=xeT[:, kc * WCH:(kc + 1) * WCH])
                he_ps = ps.tile([EH, WCH], f32, tag="psA")
                nc.tensor.matmul(out=he_ps[:], lhsT=we_s[:], rhs=xe_sb[:],
                                 start=True, stop=True)
                he_sb = sb.tile([EH, WCH], bf16, tag="hesb")
                nc.scalar.activation(out=he_sb[:], in_=he_ps[:], func=AF.Identity,
                                     bias=bec_s[:, :1], scale=1.0)
                u_ps = ps.tile([EH, WCH], f32, tag="psA")
                nc.tensor.matmul(out=u_ps[:], lhsT=w1_s[:], rhs=he_sb[:],
                                 start=True, stop=True)
                u_sb = sb.tile([EH, WCH], bf16, tag="usb")
                nc.scalar.activation(out=u_sb[:], in_=u_ps[:], func=AF.Relu,
                                     bias=b1c_s[:, :1], scale=1.0)
                for tt in range(WCH // P):
                    et = kc * (WCH // P) + tt
                    w_sb = wsp.tile([P, DD], bf16, tag="w")
                    for nb in range(DD // WCH):
                        wps = ps.tile([P, WCH], f32, tag="psA")
                        if nb % 2 == 0:
                            nc.tensor.matmul(
                                out=wps[:], lhsT=u_sb[:, tt * P:(tt + 1) * P],
                                rhs=t2p_s[:, nb * WCH:(nb + 1) * WCH],
                                start=True, stop=False)
                            nc.tensor.matmul(
                                out=wps[:], lhsT=ones_s[:, :P],
                                rhs=b2row_s[:, nb * WCH:(nb + 1) * WCH],
                                start=False, stop=True)
                            nc.scalar.copy(out=w_sb[:, nb * WCH:(nb + 1) * WCH],
                                           in_=wps[:])
                        else:
                            nc.tensor.matmul(
                                out=wps[:], lhsT=u_sb[:, tt * P:(tt + 1) * P],
                                rhs=t2p_s[:, nb * WCH:(nb + 1) * WCH],
                                start=True, stop=True)
                            nc.vector.tensor_tensor(
                                out=w_sb[:, nb * WCH:(nb + 1) * WCH], in0=wps[:],
                                in1=b2r_s[:, nb * WCH:(nb + 1) * WCH], op=OP.add)
                    TQ = T_e // 4
                    nc.sync.dma_start(
                        out=w_tabs[et // TQ][(et % TQ) * P:(et % TQ + 1) * P, :],
                        in_=w_sb[:])

            # ================= 6 MPNN layers =================
            for layer in range(min(L, _NL_)):
                for t in range(T_e if _MSG_ else 0):
                    hs4 = sb.tile([P, D], bf16, tag="hsrc4")
                    nc.gpsimd.indirect_dma_start(
                        out=hs4[:], out_offset=None, in_=hgat[:],
                        in_offset=bass.IndirectOffsetOnAxis(
                            ap=srcb_s[:, t:t + 1], axis=0))
                    hs = hs4[:, :]
                    wt = wsp.tile([P, DD], bf16, tag="w")
                    TQ = T_e // 4
                    nc.sync.dma_start(
                        out=wt[:],
                        in_=w_tabs[t // TQ][(t % TQ) * P:(t % TQ + 1) * P, :])
                    prod = prp.tile([P, DD], bf16, tag="prod")
                    nc.vector.tensor_tensor(
                        out=prod[:].rearrange("p (o i) -> p o i", o=D),
                        in0=wt[:].rearrange("p (o i) -> p o i", o=D),
                        in1=hs.unsqueeze(1).broadcast_to([P, D, D]),
                        op=OP.mult)
                    pv = prod[:].rearrange("p (o i) -> p o i", o=D)
                    t32 = prp.tile([P, D * 32], bf16, tag="t32")
                    nc.vector.tensor_tensor(
                        out=t32[:].rearrange("p (o i) -> p o i", o=D),
                        in0=pv[:, :, 0:32], in1=pv[:, :, 32:64], op=OP.add)
                    tv = t32[:].rearrange("p (o i) -> p o i", o=D)
                    t16 = prp.tile([P, D * 16], bf16, tag="t16")
                    nc.vector.tensor_tensor(
                        out=t16[:].rearrange("p (o i) -> p o i", o=D),
                        in0=tv[:, :, 0:16], in1=tv[:, :, 16:32], op=OP.add)
                    tv = t16[:].rearrange("p (o i) -> p o i", o=D)
                    t8 = prp.tile([P, D * 8], bf16, tag="t8")
                    nc.vector.tensor_tensor(
                        out=t8[:].rearrange("p (o i) -> p o i", o=D),
                        in0=tv[:, :, 0:8], in1=tv[:, :, 8:16], op=OP.add)
                    m_e = sb.tile([P, D], bf16, tag="me")
                    nc.vector.tensor_reduce(
                        out=m_e[:], in_=t8[:].rearrange("p (o i) -> p o i", o=D),
                        axis=AX.X, op=OP.add)
                    selt = sb.tile([P, P], bf16, tag="sel")
                    nc.sync.dma_start(out=selt[:], in_=selm[t * P:(t + 1) * P, :])
                    rows_ps = psg.tile([P, D], f32, tag="psB")
                    nc.tensor.matmul(out=rows_ps[:], lhsT=selt[:], rhs=m_e[:],
                                     start=True, stop=True)
                    rows = sb.tile([P, D], f32, tag="rows")
                    nc.scalar.copy(out=rows[:], in_=rows_ps[:])
                    nc.sync.dma_start(out=m_tab[t * P:(t + 1) * P, :],
                                      in_=rows[:])

                # phase A: m windows -> relu/mask -> transposed mtr (resident)
                for w in range(W if _GRU_ else 0):
                    mw = sb.tile([P, D], f32, tag="mw")
                    nc.gpsimd.indirect_dma_start(
                        out=mw[:], out_offset=None, in_=m_tab[:],
                        in_offset=bass.IndirectOffsetOnAxis(
                            ap=mwo_s[:, w:w + 1], axis=0))
                    mt_ps = psg.tile([D, P], f32, tag="psB")
                    nc.tensor.transpose(out=mt_ps[:], in_=mw[:], identity=ident[:])
                    nc.scalar.activation(out=mtr[:, w * P:(w + 1) * P],
                                         in_=mt_ps[:], func=AF.Relu)
                # phase B: batched GRU over 512-wide chunks
                off = 0
                for ch in (NCH if _GRU_ else []):
                    sl = slice(off, off + ch)
                    mch = mtr[:, sl]
                    hch = hgTb[:, sl]
                    ps_r = ps.tile([D, WCH], f32, tag="psA")
                    nc.tensor.matmul(out=ps_r[:, :ch], lhsT=wiT_s[:, 0:D],
                                     rhs=mch, start=True, stop=False)
                    nc.tensor.matmul(out=ps_r[:, :ch], lhsT=whT_s[:, 0:D],
                                     rhs=hch, start=False, stop=True)
                    r_t = sb.tile([D, WCH], f32, tag="r_t", bufs=2)
                    nc.scalar.activation(out=r_t[:, :ch], in_=ps_r[:, :ch],
                                         func=AF.Sigmoid, bias=gb_s[:, 0:1],
                                         scale=1.0)
                    ps_z = ps.tile([D, WCH], f32, tag="psA")
                    nc.tensor.matmul(out=ps_z[:, :ch], lhsT=wiT_s[:, D:2 * D],
                                     rhs=mch, start=True, stop=False)
                    nc.tensor.matmul(out=ps_z[:, :ch], lhsT=whT_s[:, D:2 * D],
                                     rhs=hch, start=False, stop=True)
                    z_t = sb.tile([D, WCH], f32, tag="z_t", bufs=2)
                    nc.scalar.activation(out=z_t[:, :ch], in_=ps_z[:, :ch],
                                         func=AF.Sigmoid, bias=gb_s[:, 1:2],
                                         scale=1.0)
                    ps_xn = ps.tile([D, WCH], f32, tag="psA")
                    nc.tensor.matmul(out=ps_xn[:, :ch], lhsT=wiT_s[:, 2 * D:3 * D],
                                     rhs=mch, start=True, stop=True)
                    gxn = sb.tile([D, WCH], f32, tag="gxn", bufs=2)
                    nc.scalar.activation(out=gxn[:, :ch], in_=ps_xn[:, :ch],
                                         func=AF.Identity, bias=gb_s[:, 2:3],
                                         scale=1.0)
                    ps_hn = ps.tile([D, WCH], f32, tag="psA")
                    nc.tensor.matmul(out=ps_hn[:, :ch], lhsT=whT_s[:, 2 * D:3 * D],
                                     rhs=hch, start=True, stop=True)
                    ghn = sb.tile([D, WCH], f32, tag="ghn", bufs=2)
                    nc.scalar.activation(out=ghn[:, :ch], in_=ps_hn[:, :ch],
                                         func=AF.Identity, bias=gb_s[:, 3:4],
                                         scale=1.0)
                    t1 = sb.tile([D, WCH], f32, tag="t1", bufs=2)
                    nc.vector.tensor_tensor(out=t1[:, :ch], in0=r_t[:, :ch],
                                            in1=ghn[:, :ch], op=OP.mult)
                    nc.vector.tensor_tensor(out=t1[:, :ch], in0=t1[:, :ch],
                                            in1=gxn[:, :ch], op=OP.add)
                    n_t = sb.tile([D, WCH], f32, tag="n_t", bufs=2)
                    nc.scalar.activation(out=n_t[:, :ch], in_=t1[:, :ch],
                                         func=AF.Tanh)
                    hgch = hgT[:, sl]
                    nc.vector.tensor_tensor(out=t1[:, :ch], in0=hgch,
                                            in1=n_t[:, :ch], op=OP.subtract)
                    nc.vector.tensor_tensor(out=t1[:, :ch], in0=z_t[:, :ch],
                                            in1=t1[:, :ch], op=OP.mult)
                    nc.vector.tensor_tensor(out=hgch, in0=t1[:, :ch],
                                            in1=n_t[:, :ch], op=OP.add)
                    nc.vector.tensor_copy(out=hgTb[:, sl], in_=hgch)
                    off += ch
                # phase C: transposed h back to row-major for sharing/readout
                for w in range(W if _GRU_ else 0):
                    hgb_w = hgTb[:, w * P:(w + 1) * P]
                    hr_ps = psg.tile([P, D], bf16, tag="psB")
                    nc.tensor.transpose(out=hr_ps[:], in_=hgb_w,
                                        identity=identb[:D, :D])
                    if layer < L - 1:
                        hr_sb = sb.tile([P, D], bf16, tag="hr_sb")
                        nc.scalar.copy(out=hr_sb[:], in_=hr_ps[:])
                        nc.sync.dma_start(out=hloc[w * P:(w + 1) * P, :],
                                          in_=hr_sb[:])
                    else:
                        nc.scalar.copy(out=h_rm[:, w * D:(w + 1) * D], in_=hr_ps[:])

                if layer < L - 1:
                    nc.gpsimd.collective_compute(
                        "AllGather", OP.bypass,
                        replica_groups=[list(range(NCORES))],
                        ins=[hloc[:].opt()], outs=[hgat[:].opt()])

            # ================= Set2Set =================
            qTb = stp.tile([D, GPB], bf16)
            rTb = stp.tile([D, GPB], bf16)
            hT0 = stp.tile([D, GPB], f32)
            cT0 = stp.tile([D, GPB], f32)
            hT1 = stp.tile([D, GPB], f32)
            cT1 = stp.tile([D, GPB], f32)
            h0b = stp.tile([D, GPB], bf16)
            h1b = stp.tile([D, GPB], bf16)
            e_sb = stp.tile([GPB, NLP], f32)
            al_b = stp.tile([GPB, NLP], bf16)
            for tl in (qTb, rTb, h0b, h1b):
                nc.scalar.dma_start(out=tl[:], in_=zb[:D, :GPB])
            for tl in (hT0, cT0, hT1, cT1):
                nc.scalar.dma_start(out=tl[:], in_=zf[:D, :GPB])

            def lstm_layer(wx_parts, wh_s, h_b, hT, cT, lb_s, out_b):
                gates = []
                for g in range(4):
                    pst = psg.tile([D, GPB], f32, tag="psB")
                    first = True
                    for (wt_s, rhs_t) in wx_parts:
                        nc.tensor.matmul(out=pst[:],
                                         lhsT=wt_s[:, g * D:(g + 1) * D],
                                         rhs=rhs_t[:], start=first, stop=False)
                        first = False
                    nc.tensor.matmul(out=pst[:], lhsT=wh_s[:, g * D:(g + 1) * D],
                                     rhs=h_b[:], start=False, stop=True)
                    fn = AF.Tanh if g == 2 else AF.Sigmoid
                    gt = sb.tile([D, GPB], f32, tag=f"lstm_g{g}")
                    nc.scalar.activation(out=gt[:], in_=pst[:], func=fn,
                                         bias=lb_s[:, g:g + 1], scale=1.0)
                    gates.append(gt)
                ig, fg, gg, og = gates
                fc = sb.tile([D, GPB], f32, tag="fc")
                nc.vector.tensor_tensor(out=fc[:], in0=fg[:], in1=cT[:], op=OP.mult)
                igg = sb.tile([D, GPB], f32, tag="igg")
                nc.vector.tensor_tensor(out=igg[:], in0=ig[:], in1=gg[:], op=OP.mult)
                nc.vector.tensor_tensor(out=cT[:], in0=fc[:], in1=igg[:], op=OP.add)
                tc_ = sb.tile([D, GPB], f32, tag="tc_")
                nc.scalar.activation(out=tc_[:], in_=cT[:], func=AF.Tanh)
                nc.vector.tensor_tensor(out=hT[:], in0=og[:], in1=tc_[:], op=OP.mult)
                nc.vector.tensor_copy(out=out_b[:], in_=hT[:])

            for it in range(ITERS if _S2S_ else 0):
                lstm_layer([(wq0_s, qTb), (wr0_s, rTb)], wh0_s, h0b, hT0, cT0,
                           lb0_s, h0b)
                lstm_layer([(wi1_s, h0b)], wh1_s, h1b, hT1, cT1, lb1_s, h1b)
                nc.vector.tensor_copy(out=qTb[:], in_=hT1[:])

                off = 0
                for ch in NCH:
                    eps = ps.tile([GPB, WCH], f32, tag="psA")
                    nc.tensor.matmul(out=eps[:, :ch], lhsT=qTb[:],
                                     rhs=hgTb[:, off:off + ch], start=True,
                                     stop=True)
                    nc.vector.tensor_tensor(out=e_sb[:, off:off + ch],
                                            in0=eps[:, :ch],
                                            in1=am_s[:, off:off + ch], op=OP.add)
                    off += ch
                nmax = sb.tile([GPB, 1], f32, tag="nmax")
                nc.vector.tensor_reduce(out=nmax[:], in_=e_sb[:], axis=AX.X,
                                        op=OP.max, negate=True)
                ssum = sb.tile([GPB, 1], f32, tag="ssum")
                nc.scalar.activation(out=e_sb[:], in_=e_sb[:], func=AF.Exp,
                                     bias=nmax[:, :1], scale=1.0,
                                     accum_out=ssum[:])
                rsum = sb.tile([GPB, 1], f32, tag="rsum")
                nc.vector.reciprocal(out=rsum[:], in_=ssum[:])
                nc.vector.tensor_scalar_mul(al_b[:], e_sb[:], rsum[:, :1])

                ro_ps = psg.tile([D, GPB], f32, tag="psB")
                for w in range(W):
                    at_ps = psg.tile([P, GPB], bf16, tag="psB")
                    nc.tensor.transpose(out=at_ps[:],
                                        in_=al_b[:, w * P:(w + 1) * P],
                                        identity=identb[:GPB, :GPB])
                    at_b = sb.tile([P, GPB], bf16, tag="at_b")
                    nc.scalar.copy(out=at_b[:], in_=at_ps[:])
                    nc.tensor.matmul(
                        out=ro_ps[:], lhsT=h_rm[:, w * D:(w + 1) * D],
                        rhs=at_b[:], start=(w == 0), stop=(w == W - 1))
                nc.vector.tensor_copy(out=rTb[:], in_=ro_ps[:])

            # ================= classifier =================
            ps1 = psg.tile([D, GPB], f32, tag="psB")
            nc.tensor.matmul(out=ps1[:], lhsT=c1q_s[:], rhs=qTb[:],
                             start=True, stop=False)
            nc.tensor.matmul(out=ps1[:], lhsT=c1r_s[:], rhs=rTb[:],
                             start=False, stop=True)
            z1b = sb.tile([D, GPB], bf16, tag="z1b")
            nc.scalar.activation(out=z1b[:], in_=ps1[:], func=AF.Relu,
                                 bias=c1b_s[:, :1], scale=1.0)
            ps2 = psg.tile([OUT, GPB], f32, tag="psB")
            nc.tensor.matmul(out=ps2[:], lhsT=c2t_s[:], rhs=z1b[:],
                             start=True, stop=True)
            yout = sb.tile([OUT, GPB], f32, tag="yout")
            nc.scalar.activation(out=yout[:], in_=ps2[:], func=AF.Identity,
                                 bias=c2b_s[:, :1], scale=1.0)
            nc.sync.dma_start(out=y[:], in_=yout[:])
        nc.__enter_lp.__exit__(None, None, None)

    _split_multi_waits(nc, mybir, bass_rust)
    return nc


# ------------------------------------------------------------------- driver --
def kernel(x_node, x_edge, params, src, dst, node2graph):
    from concourse.bass_utils import run_bass_kernel_spmd

    x_node = np.asarray(x_node, np.float32)
    x_edge = np.asarray(x_edge, np.float32)
    src = np.asarray(src, np.int32)
    dst = np.asarray(dst, np.int32)
    node2graph = np.asarray(node2graph, np.int32)
    p = {k: np.asarray(v, np.float32) for k, v in params.items()}

    plan = _plan(src, dst, node2graph)
    NLP, W, T_e, ET = plan["NLP"], plan["W"], plan["T_e"], plan["ET"]
    nsplit = plan["nsplit"]

    key = (NLP, T_e, tuple(int(v) for v in nsplit))
    if key not in _CACHE:
        _CACHE[key] = _build(plan)
    nc = _CACHE[key]

    T2p = p["en_w2"].reshape(D, D, EH).transpose(2, 1, 0).reshape(EH, DD)
    b2p = p["en_b2"].reshape(D, D).T.reshape(1, DD)
    gb = np.stack([
        p["gru_bi"][:D] + p["gru_bh"][:D],
        p["gru_bi"][D:2 * D] + p["gru_bh"][D:2 * D],
        p["gru_bi"][2 * D:],
        p["gru_bh"][2 * D:],
    ], axis=1).astype(np.float32)
    lb0 = (p["lstm_bih0"] + p["lstm_bhh0"]).reshape(4, D).T.copy().astype(np.float32)
    lb1 = (p["lstm_bih1"] + p["lstm_bhh1"]).reshape(4, D).T.copy().astype(np.float32)

    NT = NCORES * NLP
    xnp = np.zeros((NA, NT), np.float32)
    for c in range(NCORES):
        lo, hi = int(nsplit[c]), int(nsplit[c + 1])
        xnp[:, c * NLP:c * NLP + hi - lo] = x_node[lo:hi].T
    b2rep_np = np.broadcast_to(b2p, (P, DD)).copy()
    rep = {
        "xnp": xnp.astype(BF),
        "wemb": np.ascontiguousarray(p["node_emb_w"].T).astype(BF),
        "bnode_rep": np.broadcast_to(p["node_emb_b"], (P, D)).copy().astype(np.float32),
        "bnode_col": p["node_emb_b"].reshape(D, 1).astype(np.float32),
        "we_l": np.ascontiguousarray(p["edge_emb_w"].T).astype(BF),
        "be_col": p["edge_emb_b"].reshape(EH, 1).astype(np.float32),
        "w1_l": np.ascontiguousarray(p["en_w1"].T).astype(BF),
        "b1_col": p["en_b1"].reshape(EH, 1).astype(np.float32),
        "t2p": np.ascontiguousarray(T2p).astype(BF),
        "b2rep": b2rep_np.astype(BF),
        "b2row": np.ascontiguousarray(b2p).astype(BF),
        "ones_row": np.ones((1, P), BF),
        "wiT": np.ascontiguousarray(p["gru_wi"].T).astype(BF),
        "whT": np.ascontiguousarray(p["gru_wh"].T).astype(BF),
        "gbias": gb,
        "wq0": np.ascontiguousarray(p["lstm_wih0"][:, :D].T).astype(BF),
        "wr0": np.ascontiguousarray(p["lstm_wih0"][:, D:].T).astype(BF),
        "wh0": np.ascontiguousarray(p["lstm_whh0"].T).astype(BF),
        "wi1": np.ascontiguousarray(p["lstm_wih1"].T).astype(BF),
        "wh1": np.ascontiguousarray(p["lstm_whh1"].T).astype(BF),
        "lb0": lb0, "lb1": lb1,
        "c1q": np.ascontiguousarray(p["c1_w"][:, :D].T).astype(BF),
        "c1r": np.ascontiguousarray(p["c1_w"][:, D:].T).astype(BF),
        "c1b": p["c1_b"].reshape(D, 1).astype(np.float32),
        "c2t": np.ascontiguousarray(p["c2_w"].T).astype(BF),
        "c2b": p["c2_b"].reshape(OUT, 1).astype(np.float32),
        "idf": np.eye(P, dtype=np.float32),
        "idb": np.eye(P, dtype=np.float32).astype(BF),
        "zf": np.zeros((P, D), np.float32),
        "zb": np.zeros((P, D), BF),
    }

    in_maps = []
    for c in range(NCORES):
        pc = plan["cores"][c]
        lo, hi = int(nsplit[c]), int(nsplit[c + 1])
        xnl = np.zeros((NA, NLP), np.float32)
        xnl[:, :hi - lo] = x_node[lo:hi].T
        m = dict(rep)
        m["zmt"] = np.zeros((P, D), np.float32)
        m["mwoff"] = np.ascontiguousarray(pc["mpos"].reshape(W, P).T)
        m["xnl"] = xnl.astype(BF)
        m["xeT"] = np.ascontiguousarray(x_edge[pc["xe_order"]].T).astype(BF)
        m["selm"] = pc["selm"].astype(BF)
        m["srcb"] = np.ascontiguousarray(pc["src_idx"].reshape(T_e, P).T)
        m["scatb"] = np.ascontiguousarray(pc["scat"].reshape(T_e, P).T)
        m["amask"] = pc["amask"].astype(BF)
        m["vmaskb"] = np.ascontiguousarray(
            pc["vmask"].reshape(W, P).T).astype(np.float32)
        in_maps.append(m)

    res = run_bass_kernel_spmd(nc, in_maps, list(range(NCORES)))
    out = np.concatenate([res.results[c]["y"].T for c in range(NCORES)], axis=0)
    return out.astype(np.float32)
